# revision 1
# baseline (speedup 1.0000x reference)
"""Self-contained 8-core Trainium2 Bass kernel for a 2-layer GAT + linear classifier.

Strategy (dst-sharded 1D graph parallelism):
  - Host: add self-loops, compute in-degrees, sort nodes by degree (desc),
    deal round-robin to 8 cores.  Each core owns 12500 dst nodes; its nodes
    are degree-sorted so 128-node tiles have near-uniform degree.  All
    edge-index-derived tables (gather offset tables, per-tile slot counts)
    are precomputed on the host and baked into the program/inputs.
  - Device, per layer: h' = h @ W and s = h'@a_s are computed per owner shard;
    rows [h' | s] are AllGathered into a full HBM table (100001 x 65, last row
    is a sentinel with h=0, s=-1e4 used for padding slots).  Edges of each dst
    tile are gathered with one indirect DMA (260B per edge slot) into SBUF as
    [128 nodes x D_t slots x 65].  Segment softmax runs along the free axis
    (ACT for exp, DVE for max/recip), the p-weighted aggregation multiplies
    in-place on DVE (broadcast AP) and reduces over slots with PE
    identity-matmul accumulation in PSUM.
  - Output: classifier per tile, unpermuted on the host.
"""

import os
import sys
import types
from dataclasses import dataclass, field

import numpy as np

P = 128


@dataclass
class GatCfg:
    n: int = 100000
    in_dim: int = 128
    hid: int = 64
    classes: int = 40
    cores: int = 8
    slope: float = 0.2
    s_pad: float = -1.0e4
    group_slots: int = 112
    # debug staging: 1=phaseA only, 2=+AG0, 3=+edge L0, 4=full
    stage: int = 4
    use_pe_reduce: bool = False

    @property
    def shard(self):
        assert self.n % self.cores == 0
        return self.n // self.cores

    @property
    def nt(self):
        return (self.shard + P - 1) // P

    @property
    def tab(self):
        return self.n + 1


CFG = GatCfg()


def _ensure_profile_hook():
    """Synthesize antenv.axon_hooks so trace=True can capture NTFF under axon."""
    if "antenv.axon_hooks" in sys.modules:
        return
    try:
        import antenv
        mod = types.ModuleType("antenv.axon_hooks")
        mod._hook = None
        def _set(h):
            mod._hook = h
        def _get():
            return mod._hook
        mod.set_axon_ntff_profile_hook = _set
        mod.get_axon_ntff_profile_hook = _get
        sys.modules["antenv.axon_hooks"] = mod
        antenv.axon_hooks = mod
        from trn_agent_boot.trn_boot import _ntff_profile_via_ctypes
        _set(_ntff_profile_via_ctypes("/opt/axon/libaxon_pjrt.so"))
    except Exception:
        pass


# --------------------------------------------------------------------------
# Host preprocessing: sharding, tiling, gather tables
# --------------------------------------------------------------------------

@dataclass
class Prep:
    perms: list                       # per core: global node ids in local order
    D: np.ndarray                     # [nt] slots per node for each tile
    rows_t: list                      # [nt] rows per tile
    groups: list                      # (t0, n_tiles, S, rows, idx_off)
    idx_cores: list                   # per core: flat int32 gather tables
    tot_idx: int = 0


def preprocess(edge_index, cfg: GatCfg) -> Prep:
    N, NC, GS = cfg.n, cfg.cores, cfg.group_slots
    shard = cfg.shard
    src = np.asarray(edge_index[0]).astype(np.int64)
    dst = np.asarray(edge_index[1]).astype(np.int64)
    loop = np.arange(N, dtype=np.int64)
    src_all = np.concatenate([src, loop])
    dst_all = np.concatenate([dst, loop])
    deg = np.bincount(dst_all, minlength=N).astype(np.int64)
    order = np.argsort(dst_all, kind="stable")
    srcs_by_dst = src_all[order]
    rowptr = np.zeros(N + 1, np.int64)
    np.cumsum(deg, out=rowptr[1:])

    rank_order = np.argsort(-deg, kind="stable")
    table_row = np.empty(N, np.int64)
    rr = np.arange(N)
    table_row[rank_order] = (rr % NC) * shard + rr // NC
    perms = [rank_order[c::NC] for c in range(NC)]

    nt = cfg.nt
    rows_t = [min(P, shard - t * P) for t in range(nt)]
    D = np.zeros(nt, np.int64)
    for c in range(NC):
        dc = deg[perms[c]]
        for t in range(nt):
            D[t] = max(D[t], dc[t * P:t * P + rows_t[t]].max())

    groups = []
    t = 0
    idx_off = 0
    while t < nt:
        if rows_t[t] < P:
            groups.append((t, 1, int(D[t]), rows_t[t], idx_off))
            idx_off += rows_t[t] * int(D[t])
            t += 1
            continue
        S = 0
        t0 = t
        while t < nt and rows_t[t] == P and (S == 0 or S + D[t] <= GS):
            S += int(D[t])
            t += 1
        groups.append((t0, t - t0, S, P, idx_off))
        idx_off += P * S
    tot_idx = idx_off

    SENT = N
    # column-offset version of groups: (t0, ntl, S, rows, col_off)
    groups2 = []
    coff = 0
    for (t0, ntl, S, rows, _off) in groups:
        groups2.append((t0, ntl, S, rows, coff))
        coff += S
    sum_s = coff
    idx_cores = []
    for c in range(NC):
        parts = []
        for (t0, ntl, S, rows, _off) in groups:
            arr = np.full((P, S), SENT, np.int32)   # rows 84..127 padded
            off = 0
            for t2 in range(t0, t0 + ntl):
                Dt = int(D[t2])
                nodes = perms[c][t2 * P:t2 * P + rows]
                degs = deg[nodes]
                starts = rowptr[nodes]
                pos = starts[:, None] + np.arange(Dt)[None, :]
                mask = np.arange(Dt)[None, :] < degs[:, None]
                block = np.full((rows, Dt), SENT, np.int64)
                block[mask] = table_row[srcs_by_dst[pos[mask]]]
                arr[:rows, off:off + Dt] = block.astype(np.int32)
                off += Dt
            parts.append(arr)
        flat = np.ascontiguousarray(np.concatenate(parts, axis=1),
                                    dtype=np.int32).reshape(-1)
        idx_cores.append(flat)
    return Prep(perms=perms, D=D, rows_t=rows_t, groups=groups2,
                idx_cores=idx_cores, tot_idx=int(P * sum_s))


# --------------------------------------------------------------------------
# Device program
# --------------------------------------------------------------------------

def build_program(cfg: GatCfg, pp: Prep, enable_asserts=False):
    import concourse.bass as bass
    import concourse.mybir as mybir
    import concourse.tile as tile
    from concourse import bacc

    f32 = mybir.dt.float32
    i32 = mybir.dt.int32
    A = mybir.AluOpType
    AF = mybir.ActivationFunctionType
    IN, H, CLS, NC = cfg.in_dim, cfg.hid, cfg.classes, cfg.cores
    HS = H + 1
    N, NT, TAB, shard = cfg.n, cfg.nt, cfg.tab, cfg.shard
    Dmax = int(pp.D.max())
    GSmax = max(g[2] for g in pp.groups)

    nc = bacc.Bacc("TRN2", target_bir_lowering=False, debug=False,
                   enable_asserts=enable_asserts, num_devices=NC)

    x_t = nc.dram_tensor("xT_shard", [IN, shard], f32, kind="ExternalInput")
    idx_t = nc.dram_tensor("idx_flat", [pp.tot_idx], i32, kind="ExternalInput")
    W0_t = nc.dram_tensor("W0", [IN, H], f32, kind="ExternalInput")
    W1_t = nc.dram_tensor("W1", [H, H], f32, kind="ExternalInput")
    Wl_t = nc.dram_tensor("Wl", [H, CLS], f32, kind="ExternalInput")
    asb0_t = nc.dram_tensor("asb0", [P, H], f32, kind="ExternalInput")
    adb0_t = nc.dram_tensor("adb0", [P, H], f32, kind="ExternalInput")
    asb1_t = nc.dram_tensor("asb1", [P, H], f32, kind="ExternalInput")
    adb1_t = nc.dram_tensor("adb1", [P, H], f32, kind="ExternalInput")
    b0b_t = nc.dram_tensor("b0b", [P, H], f32, kind="ExternalInput")
    b1b_t = nc.dram_tensor("b1b", [P, H], f32, kind="ExternalInput")
    blb_t = nc.dram_tensor("blb", [P, CLS], f32, kind="ExternalInput")
    ident_t = nc.dram_tensor("ident", [P, P], f32, kind="ExternalInput")
    sent_t = nc.dram_tensor("sentrow", [1, HS], f32, kind="ExternalInput")
    y_t = nc.dram_tensor("y_out", [shard, CLS], f32, kind="ExternalOutput")

    ha0_loc = nc.dram_tensor("ha0_loc", [shard, HS], f32, kind="Internal")
    ha1_loc = nc.dram_tensor("ha1_loc", [shard, HS], f32, kind="Internal")
    ha0_full = nc.dram_tensor("ha0_full", [TAB, HS], f32, kind="Internal")
    ha1_full = nc.dram_tensor("ha1_full", [TAB, HS], f32, kind="Internal")

    with tile.TileContext(nc) as tc:
        with tc.tile_pool(name="const", bufs=1) as cp, \
             tc.tile_pool(name="gp", bufs=4) as gp, \
             tc.tile_pool(name="wp", bufs=3) as wp, \
             tc.tile_pool(name="ps", bufs=2, space="PSUM") as ps:

            def load_const(t, shape):
                s = cp.tile(shape, f32, tag=f"c_{t.name}")
                nc.sync.dma_start(s[:], t.ap())
                return s

            W0_s = load_const(W0_t, [IN, H])
            W1_s = load_const(W1_t, [H, H])
            Wl_s = load_const(Wl_t, [H, CLS])
            asb0_s = load_const(asb0_t, [P, H])
            adb0_s = load_const(adb0_t, [P, H])
            asb1_s = load_const(asb1_t, [P, H])
            adb1_s = load_const(adb1_t, [P, H])
            b0b_s = load_const(b0b_t, [P, H])
            b1b_s = load_const(b1b_t, [P, H])
            blb_s = load_const(blb_t, [P, CLS])
            ident_s = load_const(ident_t, [P, P])
            sent_s = load_const(sent_t, [1, HS])

            SUMS = pp.tot_idx // P
            idx_all = cp.tile([P, SUMS], i32)
            nc.sync.dma_start(
                idx_all[:],
                idx_t.ap()[:].rearrange("(p s) -> p s", s=SUMS))
            d0_all = cp.tile([P, NT], f32)
            d0s_all = cp.tile([P, NT], f32)
            d1_all = cp.tile([P, NT], f32)
            d1s_all = cp.tile([P, NT], f32)
            junk = cp.tile([P, H], f32)

            # sentinel rows of both tables
            nc.sync.dma_start(ha0_full.ap()[N:N + 1, :], sent_s[:])
            nc.sync.dma_start(ha1_full.ap()[N:N + 1, :], sent_s[:])

            # ---------------- phase A: h0' = x @ W0, s0, d0 ----------------
            xT_all = cp.tile([IN, shard], f32)
            nc.sync.dma_start(xT_all[:], x_t.ap())
            for t in range(NT):
                rows = pp.rows_t[t]
                h0_p = ps.tile([P, H], f32, space="PSUM", tag="mm")
                nc.tensor.matmul(out=h0_p[:rows, :],
                                 lhsT=xT_all[:, t * P:t * P + rows],
                                 rhs=W0_s[:], start=True, stop=True)
                ha_tile = wp.tile([rows, HS], f32, tag="hat")
                nc.scalar.copy(ha_tile[:, :H], h0_p[:rows, :])
                nc.vector.tensor_tensor(out=junk[:rows, :], in0=h0_p[:rows, :],
                                        in1=asb0_s[:rows, :], op=A.mult)
                nc.vector.tensor_reduce(out=ha_tile[:, H:HS], in_=junk[:rows, :],
                                        axis=mybir.AxisListType.X, op=A.add)
                nc.vector.tensor_tensor(out=junk[:rows, :], in0=h0_p[:rows, :],
                                        in1=adb0_s[:rows, :], op=A.mult)
                nc.vector.tensor_reduce(out=d0_all[:rows, t:t + 1],
                                        in_=junk[:rows, :],
                                        axis=mybir.AxisListType.X, op=A.add)
                nc.sync.dma_start(ha0_loc.ap()[t * P:t * P + rows, :], ha_tile[:])
            nc.vector.tensor_scalar(d0s_all[:], d0_all[:], cfg.slope, None,
                                    op0=A.mult)

            if cfg.stage >= 2:
                nc.gpsimd.collective_compute(
                    "AllGather", A.bypass,
                    replica_groups=[list(range(NC))],
                    ins=[ha0_loc.ap()],
                    outs=[ha0_full.ap()[0:N, :]],
                )

            # ---------------- edge phase ----------------
            def edge_phase(table, d_all, ds_all, post):
                for (t0, ntl, S, rows, col_off) in pp.groups:
                    G = gp.tile([rows, S, HS], f32, tag="G")
                    # HW indirect DMA consumes ONE offset per partition per
                    # call (sim's multi-offset form diverges on silicon), so
                    # gather one slot-column (<=128 rows) at a time.
                    for j in range(S):
                        nc.gpsimd.indirect_dma_start(
                            out=G[:, j, :], out_offset=None,
                            in_=table.ap(),
                            in_offset=bass.IndirectOffsetOnAxis(
                                ap=idx_all[:rows, col_off + j:col_off + j + 1],
                                axis=0))
                    off = 0
                    for t in range(t0, t0 + ntl):
                        Dt = int(pp.D[t])
                        Gt = G[:, off:off + Dt, :]
                        off += Dt
                        sG = Gt[:, :, H]
                        z = wp.tile([rows, Dt], f32, tag="z")
                        nc.scalar.activation(z[:], sG, AF.Identity,
                                             bias=d_all[:rows, t:t + 1], scale=1.0)
                        t02 = wp.tile([rows, Dt], f32, tag="t02")
                        nc.scalar.activation(t02[:], sG, AF.Identity,
                                             bias=ds_all[:rows, t:t + 1],
                                             scale=cfg.slope)
                        nc.vector.tensor_tensor(out=z[:], in0=z[:], in1=t02[:],
                                                op=A.max)
                        nm = wp.tile([rows, 1], f32, tag="nm")
                        nc.vector.tensor_reduce(out=nm[:], in_=z[:],
                                                axis=mybir.AxisListType.X,
                                                op=A.max, negate=True)
                        p_t = wp.tile([rows, Dt], f32, tag="pt")
                        den = wp.tile([rows, 1], f32, tag="den")
                        nc.scalar.activation(p_t[:], z[:], AF.Exp, bias=nm[:],
                                             scale=1.0, accum_out=den[:])
                        r_t = wp.tile([rows, 1], f32, tag="rt")
                        nc.vector.reciprocal(r_t[:], den[:])
                        nc.vector.tensor_tensor(
                            out=Gt[:, :, 0:H], in0=Gt[:, :, 0:H],
                            in1=p_t[:].to_broadcast([rows, Dt, H]), op=A.mult)
                        hagg = wp.tile([rows, H], f32, tag="hagg")
                        if cfg.use_pe_reduce:
                            agg_p = ps.tile([P, H], f32, space="PSUM", tag="agg")
                            for d in range(Dt):
                                nc.tensor.matmul(out=agg_p[:rows, :],
                                                 lhsT=ident_s[:rows, :rows],
                                                 rhs=Gt[:, d, 0:H],
                                                 start=(d == 0),
                                                 stop=(d == Dt - 1))
                            nc.vector.tensor_scalar(hagg[:], agg_p[:rows, :],
                                                    r_t[:], None, op0=A.mult)
                        else:
                            agg_s = wp.tile([rows, H], f32, tag="aggs")
                            nc.vector.tensor_reduce(
                                out=agg_s[:],
                                in_=Gt[:, :, 0:H].rearrange("p d f -> p f d"),
                                axis=mybir.AxisListType.X, op=A.add)
                            nc.vector.tensor_scalar(hagg[:], agg_s[:],
                                                    r_t[:], None, op0=A.mult)
                        bias_s = b0b_s if table is ha0_full else b1b_s
                        nc.vector.tensor_tensor(out=hagg[:], in0=hagg[:],
                                                in1=bias_s[:rows, :], op=A.add)
                        # ELU = relu(x) + min(exp(x)-1, 0)
                        ex = wp.tile([rows, H], f32, tag="ex")
                        nc.scalar.activation(ex[:], hagg[:], AF.Exp)
                        nc.vector.tensor_scalar(ex[:], ex[:], -1.0, 0.0,
                                                op0=A.add, op1=A.min)
                        rl = wp.tile([rows, H], f32, tag="rl")
                        nc.vector.tensor_scalar(rl[:], hagg[:], 0.0, None,
                                                op0=A.max)
                        h_t = wp.tile([rows, H], f32, tag="ht")
                        nc.vector.tensor_tensor(out=h_t[:], in0=rl[:], in1=ex[:],
                                                op=A.add)
                        post(t, h_t, rows)

            # ---------------- layer-0 post: h1' = elu_out @ W1, s1, d1 -----
            def post_l0(t, h_t, rows):
                hT_p = ps.tile([H, P], f32, space="PSUM", tag="tp")
                nc.tensor.transpose(out=hT_p[:, :rows], in_=h_t[:],
                                    identity=ident_s[:rows, :rows])
                hT_s = wp.tile([H, P], f32, tag="hts")
                nc.vector.tensor_copy(out=hT_s[:, :rows], in_=hT_p[:, :rows])
                h1_p = ps.tile([P, H], f32, space="PSUM", tag="mm")
                nc.tensor.matmul(out=h1_p[:rows, :], lhsT=hT_s[:, :rows],
                                 rhs=W1_s[:], start=True, stop=True)
                ha_tile = wp.tile([rows, HS], f32, tag="hat")
                nc.scalar.copy(ha_tile[:, :H], h1_p[:rows, :])
                nc.vector.tensor_tensor(out=junk[:rows, :], in0=h1_p[:rows, :],
                                        in1=asb1_s[:rows, :], op=A.mult)
                nc.vector.tensor_reduce(out=ha_tile[:, H:HS], in_=junk[:rows, :],
                                        axis=mybir.AxisListType.X, op=A.add)
                nc.vector.tensor_tensor(out=junk[:rows, :], in0=h1_p[:rows, :],
                                        in1=adb1_s[:rows, :], op=A.mult)
                nc.vector.tensor_reduce(out=d1_all[:rows, t:t + 1],
                                        in_=junk[:rows, :],
                                        axis=mybir.AxisListType.X, op=A.add)
                nc.sync.dma_start(ha1_loc.ap()[t * P:t * P + rows, :], ha_tile[:])

            if cfg.stage >= 3:
                edge_phase(ha0_full, d0_all, d0s_all, post_l0)
            if cfg.stage >= 4:
                nc.vector.tensor_scalar(d1s_all[:], d1_all[:], cfg.slope, None,
                                        op0=A.mult)
                nc.gpsimd.collective_compute(
                    "AllGather", A.bypass,
                    replica_groups=[list(range(NC))],
                    ins=[ha1_loc.ap()],
                    outs=[ha1_full.ap()[0:N, :]],
                )

            # ---------------- layer-1 post: classifier ----------------
            def post_l1(t, h_t, rows):
                hT_p = ps.tile([H, P], f32, space="PSUM", tag="tp")
                nc.tensor.transpose(out=hT_p[:, :rows], in_=h_t[:],
                                    identity=ident_s[:rows, :rows])
                hT_s = wp.tile([H, P], f32, tag="hts")
                nc.vector.tensor_copy(out=hT_s[:, :rows], in_=hT_p[:, :rows])
                y_p = ps.tile([P, CLS], f32, space="PSUM", tag="mm")
                nc.tensor.matmul(out=y_p[:rows, :], lhsT=hT_s[:, :rows],
                                 rhs=Wl_s[:], start=True, stop=True)
                y_s = wp.tile([rows, CLS], f32, tag="ys")
                nc.vector.tensor_tensor(out=y_s[:], in0=y_p[:rows, :],
                                        in1=blb_s[:rows, :], op=A.add)
                nc.sync.dma_start(y_t.ap()[t * P:t * P + rows, :], y_s[:])

            if cfg.stage >= 4:
                edge_phase(ha1_full, d1_all, d1s_all, post_l1)
            else:
                for t in range(NT):
                    rows = pp.rows_t[t]
                    nc.sync.dma_start(y_t.ap()[t * P:t * P + rows, :],
                                      blb_s[:rows, :])

    nc.compile()
    return nc


def make_in_maps(cfg: GatCfg, pp: Prep, x, W0, a_s0, a_d0, b0, W1, a_s1, a_d1,
                 b1, Wl, bl):
    x = np.asarray(x, np.float32)
    consts = dict(
        W0=np.ascontiguousarray(W0, np.float32),
        W1=np.ascontiguousarray(W1, np.float32),
        Wl=np.ascontiguousarray(Wl, np.float32),
        asb0=np.ascontiguousarray(np.tile(np.asarray(a_s0, np.float32)[None, :], (P, 1))),
        adb0=np.ascontiguousarray(np.tile(np.asarray(a_d0, np.float32)[None, :], (P, 1))),
        asb1=np.ascontiguousarray(np.tile(np.asarray(a_s1, np.float32)[None, :], (P, 1))),
        adb1=np.ascontiguousarray(np.tile(np.asarray(a_d1, np.float32)[None, :], (P, 1))),
        b0b=np.ascontiguousarray(np.tile(np.asarray(b0, np.float32)[None, :], (P, 1))),
        b1b=np.ascontiguousarray(np.tile(np.asarray(b1, np.float32)[None, :], (P, 1))),
        blb=np.ascontiguousarray(np.tile(np.asarray(bl, np.float32)[None, :], (P, 1))),
        ident=np.eye(P, dtype=np.float32),
        sentrow=np.ascontiguousarray(
            np.concatenate([np.zeros(cfg.hid, np.float32),
                            np.array([cfg.s_pad], np.float32)])[None, :]),
    )
    in_maps = []
    for c in range(cfg.cores):
        m = dict(consts)
        m["xT_shard"] = np.ascontiguousarray(x[pp.perms[c]].T, np.float32)
        m["idx_flat"] = pp.idx_cores[c]
        in_maps.append(m)
    return in_maps


def assemble_output(cfg: GatCfg, pp: Prep, results):
    out = np.zeros((cfg.n, cfg.classes), np.float32)
    for c in range(cfg.cores):
        out[pp.perms[c]] = results[c]["y_out"]
    return out


_cache = {}
last_result = None


def kernel(**inputs) -> np.ndarray:
    global last_result
    cfg = CFG
    trace = bool(int(os.environ.get("GAT_TRACE", "0")))
    if trace:
        _ensure_profile_hook()
    from concourse.bass_utils import run_bass_kernel_spmd

    ei = np.asarray(inputs["edge_index"])
    key = hash(ei.tobytes())
    if key not in _cache:
        pp = preprocess(ei, cfg)
        nc = build_program(cfg, pp)
        _cache[key] = (pp, nc)
    pp, nc = _cache[key]

    in_maps = make_in_maps(
        cfg, pp, inputs["x"], inputs["W0"], inputs["a_s0"], inputs["a_d0"],
        inputs["b0"], inputs["W1"], inputs["a_s1"], inputs["a_d1"],
        inputs["b1"], inputs["Wl"], inputs["bl"])
    res = run_bass_kernel_spmd(nc, in_maps, core_ids=list(range(cfg.cores)),
                               trace=trace)
    last_result = res
    return assemble_output(cfg, pp, res.results)



# revision 5
# speedup vs baseline: 1.5351x; 1.5351x over previous
"""Self-contained 8-core Trainium2 Bass kernel for a 2-layer GAT + linear classifier.

v2 design (dst-sharded 1D graph parallelism):
  - Host: add self-loops, degree-sort nodes, deal round-robin to 8 cores.
    Tiles of 128 dst nodes; uniform-degree groups of tiles (all tiles in a
    group share slot width D) so softmax reductions batch into single
    strided DVE instructions.  Pad slots handled by a -3e4 additive mask.
  - Layer 0 needs NO device gather and NO AllGather: the host pre-gathers
    x[src] per edge slot (bf16, transposed per 128-slot column) and the
    device computes [h|s|d] per slot with one PE matmul per column against
    a fused rhs [W0 | W0@a_s0 | W0@a_d0].
  - Layer 1: per-tile outputs [h1|s1|d1] = elu(agg0)@[W1|W1@a_s1|W1@a_d1]
    are written to a packed bf16 table [100000, 66], AllGathered (13 MB),
    then edge rows fetched with per-column indirect DMAs (the only
    HW-correct indexed-DMA form: one offset per partition per call).
  - Classifier fused per tile; one bulk DMA for the table and the output.
"""

import os
import sys
import types
from dataclasses import dataclass

import numpy as np
import ml_dtypes

P = 128
N = 100000
IN_DIM = 128
HID = 64
CLS = 40
NC = 8
HS = 66                      # table/slot row: [h(64) | s | d-or-junk]
SHARD = N // NC              # 12500
NT = (SHARD + P - 1) // P    # 98
NTP = NT * P                 # 12544
GS = 96                      # max slots per group
SLOPE = 0.2
MASKV = -30000.0


def _ensure_profile_hook():
    if "antenv.axon_hooks" in sys.modules:
        return
    try:
        import antenv
        mod = types.ModuleType("antenv.axon_hooks")
        mod._hook = None
        def _set(h):
            mod._hook = h
        def _get():
            return mod._hook
        mod.set_axon_ntff_profile_hook = _set
        mod.get_axon_ntff_profile_hook = _get
        sys.modules["antenv.axon_hooks"] = mod
        antenv.axon_hooks = mod
        from trn_agent_boot.trn_boot import _ntff_profile_via_ctypes
        _set(_ntff_profile_via_ctypes("/opt/axon/libaxon_pjrt.so"))
    except Exception:
        pass


# --------------------------------------------------------------------------
# Host preprocessing
# --------------------------------------------------------------------------

@dataclass
class Prep:
    perms: list          # per core: global node ids in local (row) order
    groups: list         # (t0, ntl, D, coff)
    C: int               # total slot columns
    srcs: list           # per core: [P, C] int64 src node per slot (pads=0)
    idx: list            # per core: [P, C] int32 table row per slot
    mask: list           # per core: [P, C] float32 0 / MASKV
    table_row: np.ndarray


def preprocess(edge_index) -> Prep:
    src = np.asarray(edge_index[0]).astype(np.int64)
    dst = np.asarray(edge_index[1]).astype(np.int64)
    loop = np.arange(N, dtype=np.int64)
    src_all = np.concatenate([src, loop])
    dst_all = np.concatenate([dst, loop])
    deg = np.bincount(dst_all, minlength=N).astype(np.int64)
    order = np.argsort(dst_all, kind="stable")
    srcs_by_dst = src_all[order]
    rowptr = np.zeros(N + 1, np.int64)
    np.cumsum(deg, out=rowptr[1:])

    rank_order = np.argsort(-deg, kind="stable")
    table_row = np.empty(N, np.int64)
    rr = np.arange(N)
    table_row[rank_order] = (rr % NC) * SHARD + rr // NC
    perms = [rank_order[c::NC] for c in range(NC)]

    rows_t = [min(P, SHARD - t * P) for t in range(NT)]
    D = np.zeros(NT, np.int64)
    for c in range(NC):
        dc = deg[perms[c]]
        for t in range(NT):
            D[t] = max(D[t], dc[t * P:t * P + rows_t[t]].max())

    groups = []
    t = 0
    coff = 0
    while t < NT:
        d = int(D[t])
        t0 = t
        t += 1
        while t < NT and int(D[t]) == d and (t - t0 + 1) * d <= GS:
            t += 1
        groups.append((t0, t - t0, d, coff))
        coff += (t - t0) * d
    C = coff

    srcs_l, idx_l, mask_l = [], [], []
    for c in range(NC):
        srcs = np.zeros((P, C), np.int64)
        mask = np.full((P, C), MASKV, np.float32)
        for (t0, ntl, d, goff) in groups:
            for ti in range(ntl):
                t = t0 + ti
                co = goff + ti * d
                rows = rows_t[t]
                nodes = perms[c][t * P:t * P + rows]
                degs = deg[nodes]
                starts = rowptr[nodes]
                pos = starts[:, None] + np.arange(d)[None, :]
                valid = np.arange(d)[None, :] < degs[:, None]
                blk = np.zeros((rows, d), np.int64)
                blk[valid] = srcs_by_dst[np.minimum(pos, rowptr[-1] - 1)[valid]]
                srcs[:rows, co:co + d] = blk
                m = np.full((rows, d), MASKV, np.float32)
                m[valid] = 0.0
                mask[:rows, co:co + d] = m
        srcs_l.append(srcs)
        idx_l.append(table_row[srcs].astype(np.int32))
        mask_l.append(mask)
    return Prep(perms=perms, groups=groups, C=C, srcs=srcs_l, idx=idx_l,
                mask=mask_l, table_row=table_row)


# --------------------------------------------------------------------------
# Device program
# --------------------------------------------------------------------------

def build_program(pp: Prep):
    import concourse.bass as bass
    import concourse.mybir as mybir
    import concourse.tile as tile
    from concourse import bacc

    f32 = mybir.dt.float32
    bf16 = mybir.dt.bfloat16
    i32 = mybir.dt.int32
    A = mybir.AluOpType
    AF = mybir.ActivationFunctionType
    C = pp.C
    NTLMAX = max(g[1] for g in pp.groups)

    nc = bacc.Bacc("TRN2", target_bir_lowering=False, debug=False,
                   num_devices=NC)

    xgT_t = nc.dram_tensor("xgT", [P, C * P], bf16, kind="ExternalInput")
    xTo_t = nc.dram_tensor("xTo", [P, NTP], bf16, kind="ExternalInput")
    idx_t = nc.dram_tensor("idx", [P, C], i32, kind="ExternalInput")
    mask_t = nc.dram_tensor("mask", [P, C], bf16, kind="ExternalInput")
    rhs0_t = nc.dram_tensor("rhs0", [IN_DIM, HS], bf16, kind="ExternalInput")
    rhs1_t = nc.dram_tensor("rhs1", [HID, HS], bf16, kind="ExternalInput")
    rhsl_t = nc.dram_tensor("rhsl", [HID, CLS], bf16, kind="ExternalInput")
    b0r_t = nc.dram_tensor("b0r", [P, NTLMAX * HID], bf16, kind="ExternalInput")
    b1r_t = nc.dram_tensor("b1r", [P, NTLMAX * HID], bf16, kind="ExternalInput")
    blr_t = nc.dram_tensor("blr", [P, CLS], f32, kind="ExternalInput")
    id_t = nc.dram_tensor("ident", [P, P], bf16, kind="ExternalInput")
    y_t = nc.dram_tensor("y_out", [NTP, CLS], f32, kind="ExternalOutput")

    ha1_loc = nc.dram_tensor("ha1_loc", [NTP, HS], bf16, kind="Internal")
    ha1_full = nc.dram_tensor("ha1_full", [N, HS], bf16, kind="Internal",
                              addr_space="Shared")

    with tile.TileContext(nc) as tc:
        with tc.tile_pool(name="const", bufs=1) as cp, \
             tc.tile_pool(name="xgp", bufs=2) as xgp, \
             tc.tile_pool(name="gp", bufs=2) as gp, \
             tc.tile_pool(name="wp", bufs=3) as wp, \
             tc.tile_pool(name="pmm", bufs=3, space="PSUM") as pmm, \
             tc.tile_pool(name="ptr", bufs=2, space="PSUM") as ptr:

            def load_const(t, shape, dt):
                s = cp.tile(shape, dt, tag=f"c_{t.name}")
                nc.sync.dma_start(s[:], t.ap())
                return s

            rhs0_s = load_const(rhs0_t, [IN_DIM, HS], bf16)
            rhs1_s = load_const(rhs1_t, [HID, HS], bf16)
            rhsl_s = load_const(rhsl_t, [HID, CLS], bf16)
            b0r_s = load_const(b0r_t, [P, NTLMAX * HID], bf16)
            b1r_s = load_const(b1r_t, [P, NTLMAX * HID], bf16)
            blr_s = load_const(blr_t, [P, CLS], f32)
            ident_s = load_const(id_t, [P, P], bf16)
            idx_all = load_const(idx_t, [P, C], i32)
            mask_all = load_const(mask_t, [P, C], bf16)
            xTo_s = load_const(xTo_t, [P, NTP], bf16)

            ha1_sb = cp.tile([P, NT, HS], bf16)
            y_sb = cp.tile([P, NT, CLS], f32)
            d0_all = cp.tile([P, NT], f32)
            d1_all = cp.tile([P, NT], f32)

            # ---------------- phase A: d0 for own dst nodes ----------------
            for tb in range(0, NT, 7):
                tn = min(7, NT - tb)
                mm = pmm.tile([P, 7, HS], f32, space="PSUM", tag="mmG")
                for t2 in range(tn):
                    t = tb + t2
                    nc.tensor.matmul(out=mm[:, t2, :],
                                     lhsT=xTo_s[:, t * P:(t + 1) * P],
                                     rhs=rhs0_s[:], start=True, stop=True)
                nc.vector.tensor_copy(out=d0_all[:, tb:tb + tn],
                                      in_=mm[:, 0:tn, 65])

            # ---------------- edge phase (shared for both layers) ----------
            def edge_phase(layer, d_all, post):
                for (t0, ntl, D, goff) in pp.groups:
                    S = ntl * D
                    G = gp.tile([P, S, HS], bf16, tag="G")
                    if layer == 0:
                        xg = xgp.tile([P, S * P], bf16, tag="xg")
                        nc.sync.dma_start(
                            xg[:], xgT_t.ap()[:, goff * P:(goff + S) * P])
                        for jb in range(0, S, 7):
                            jn = min(7, S - jb)
                            mm = pmm.tile([P, 7, HS], f32, space="PSUM",
                                          tag="mmG")
                            for j2 in range(jn):
                                j = jb + j2
                                nc.tensor.matmul(
                                    out=mm[:, j2, :],
                                    lhsT=xg[:, j * P:(j + 1) * P],
                                    rhs=rhs0_s[:], start=True, stop=True)
                            nc.scalar.copy(G[:, jb:jb + jn, :],
                                           mm[:, 0:jn, :])
                    else:
                        for j in range(S):
                            nc.gpsimd.indirect_dma_start(
                                out=G[:, j, :], out_offset=None,
                                in_=ha1_full.ap(),
                                in_offset=bass.IndirectOffsetOnAxis(
                                    ap=idx_all[:, goff + j:goff + j + 1],
                                    axis=0))

                    # ---- segment softmax over each tile's D slots ----
                    dmap = wp.tile([P, S], bf16, tag="dmap")
                    for i in range(ntl):
                        nc.vector.tensor_copy(
                            out=dmap[:, i * D:(i + 1) * D],
                            in_=d_all[:, t0 + i:t0 + i + 1].to_broadcast(
                                [P, D]))
                    z = wp.tile([P, S], bf16, tag="z")
                    nc.vector.tensor_tensor(out=z[:], in0=G[:, :, 64],
                                            in1=dmap[:], op=A.add)
                    zs = wp.tile([P, S], bf16, tag="zs")
                    nc.vector.tensor_scalar(zs[:], z[:], SLOPE, None,
                                            op0=A.mult)
                    nc.vector.tensor_tensor(out=z[:], in0=z[:], in1=zs[:],
                                            op=A.max)
                    nc.vector.tensor_tensor(out=z[:], in0=z[:],
                                            in1=mask_all[:, goff:goff + S],
                                            op=A.add)
                    zv = z[:].rearrange("p (t d) -> p t d", t=ntl)
                    nm = wp.tile([P, ntl], bf16, tag="nm")
                    nc.vector.tensor_reduce(out=nm[:], in_=zv,
                                            axis=mybir.AxisListType.X,
                                            op=A.max, negate=True)
                    nc.vector.tensor_tensor(
                        out=zv, in0=zv,
                        in1=nm[:].to_broadcast([P, ntl, D]), op=A.add)
                    nc.scalar.activation(z[:], z[:], AF.Exp)
                    den = wp.tile([P, ntl], f32, tag="den")
                    nc.vector.tensor_reduce(out=den[:], in_=zv,
                                            axis=mybir.AxisListType.X,
                                            op=A.add)
                    rden = wp.tile([P, ntl], f32, tag="rden")
                    nc.vector.reciprocal(rden[:], den[:])
                    rb = wp.tile([P, ntl], bf16, tag="rb")
                    nc.vector.tensor_copy(out=rb[:], in_=rden[:])
                    nc.vector.tensor_tensor(
                        out=zv, in0=zv,
                        in1=rb[:].to_broadcast([P, ntl, D]), op=A.mult)
                    # ---- weighted aggregation: G *= alpha, tree-reduce ----
                    nc.vector.tensor_tensor(
                        out=G[:, :, 0:HID], in0=G[:, :, 0:HID],
                        in1=z[:].to_broadcast([P, S, HID]), op=A.mult)
                    G4 = G[:, :, :].rearrange("p (t d) e -> p t d e", t=ntl)
                    h = D
                    while h > 1:
                        a = (h + 1) // 2
                        nc.vector.tensor_tensor(
                            out=G4[:, :, 0:h - a, 0:HID],
                            in0=G4[:, :, 0:h - a, 0:HID],
                            in1=G4[:, :, a:h, 0:HID], op=A.add)
                        h = a
                    agg = G4[:, :, 0, 0:HID]          # [P, ntl, HID]
                    br = (b0r_s if layer == 0 else b1r_s)
                    hb = wp.tile([P, ntl * HID], bf16, tag="hb")
                    nc.vector.tensor_tensor(
                        out=hb[:].rearrange("p (t e) -> p t e", t=ntl),
                        in0=agg,
                        in1=br[:, 0:ntl * HID].rearrange(
                            "p (t e) -> p t e", t=ntl), op=A.add)
                    ex = wp.tile([P, ntl * HID], bf16, tag="ex")
                    nc.scalar.activation(ex[:], hb[:], AF.Exp)
                    nc.vector.tensor_scalar(ex[:], ex[:], -1.0, 0.0,
                                            op0=A.add, op1=A.min)
                    rl = wp.tile([P, ntl * HID], bf16, tag="rl")
                    nc.vector.tensor_scalar(rl[:], hb[:], 0.0, None,
                                            op0=A.max)
                    ht = wp.tile([P, ntl * HID], bf16, tag="ht")
                    nc.vector.tensor_tensor(out=ht[:], in0=rl[:], in1=ex[:],
                                            op=A.add)
                    post(t0, ntl, ht)

            # ---------------- posts ----------------
            def transpose_pairs(ntl, ht, consume):
                for i in range(ntl):
                    tp = ptr.tile([HID, P], bf16, space="PSUM", tag="tp")
                    nc.tensor.transpose(out=tp[:, :],
                                        in_=ht[:, i * HID:(i + 1) * HID],
                                        identity=ident_s[:])
                    hT2 = wp.tile([HID, P], bf16, tag="hT2")
                    nc.scalar.copy(hT2[:, :], tp[:, :])
                    consume(i, hT2[:, :])

            def post_l0(t0, ntl, ht):
                def consume(i2, lhsT):
                    t = t0 + i2
                    mm = pmm.tile([P, HS], f32, space="PSUM", tag="mmP")
                    nc.tensor.matmul(out=mm[:, :], lhsT=lhsT, rhs=rhs1_s[:],
                                     start=True, stop=True)
                    nc.scalar.copy(ha1_sb[:, t, :], mm[:, :])
                    nc.vector.tensor_copy(out=d1_all[:, t:t + 1],
                                          in_=mm[:, 65:66])
                transpose_pairs(ntl, ht, consume)

            def post_l1(t0, ntl, ht):
                def consume(i2, lhsT):
                    t = t0 + i2
                    mm = pmm.tile([P, HS], f32, space="PSUM", tag="mmP")
                    nc.tensor.matmul(out=mm[:, 0:CLS], lhsT=lhsT, rhs=rhsl_s[:],
                                     start=True, stop=True)
                    nc.vector.tensor_tensor(out=y_sb[:, t, :], in0=mm[:, 0:CLS],
                                            in1=blr_s[:], op=A.add)
                transpose_pairs(ntl, ht, consume)

            # ---------------- run ----------------
            edge_phase(0, d0_all, post_l0)
            nc.sync.dma_start(
                ha1_loc.ap().rearrange("(t p) e -> p t e", p=P),
                ha1_sb[:, :, :])
            nc.gpsimd.collective_compute(
                "AllGather", A.bypass,
                replica_groups=[list(range(NC))],
                ins=[ha1_loc.ap()[0:SHARD, :]],
                outs=[ha1_full.ap()[0:N, :]],
            )
            edge_phase(1, d1_all, post_l1)
            nc.sync.dma_start(
                y_t.ap().rearrange("(t p) e -> p t e", p=P),
                y_sb[:, :, :])

    nc.compile()
    return nc


# --------------------------------------------------------------------------
# Input staging / output assembly
# --------------------------------------------------------------------------

def make_in_maps(pp: Prep, x, W0, a_s0, a_d0, b0, W1, a_s1, a_d1, b1, Wl, bl):
    bf = ml_dtypes.bfloat16
    x = np.asarray(x, np.float32)
    W0 = np.asarray(W0, np.float32)
    W1 = np.asarray(W1, np.float32)
    Wl = np.asarray(Wl, np.float32)
    NTLMAX = max(g[1] for g in pp.groups)

    rhs0 = np.concatenate(
        [W0, (W0 @ np.asarray(a_s0, np.float32))[:, None],
         (W0 @ np.asarray(a_d0, np.float32))[:, None]], axis=1)
    rhs1 = np.concatenate(
        [W1, (W1 @ np.asarray(a_s1, np.float32))[:, None],
         (W1 @ np.asarray(a_d1, np.float32))[:, None]], axis=1)
    consts = dict(
        rhs0=np.ascontiguousarray(rhs0).astype(bf),
        rhs1=np.ascontiguousarray(rhs1).astype(bf),
        rhsl=np.ascontiguousarray(Wl).astype(bf),
        b0r=np.ascontiguousarray(
            np.tile(np.asarray(b0, np.float32)[None, :], (P, NTLMAX))).astype(bf),
        b1r=np.ascontiguousarray(
            np.tile(np.asarray(b1, np.float32)[None, :], (P, NTLMAX))).astype(bf),
        blr=np.ascontiguousarray(
            np.tile(np.asarray(bl, np.float32)[None, :], (P, 1))),
        ident=np.eye(P, dtype=np.float32).astype(bf),
    )
    xb = x.astype(bf)
    in_maps = []
    for c in range(NC):
        m = dict(consts)
        xg = xb[pp.srcs[c]]                       # [P, C, IN_DIM]
        m["xgT"] = np.ascontiguousarray(
            xg.transpose(2, 1, 0).reshape(IN_DIM, pp.C * P))
        xTo = np.zeros((P, NTP), np.float32)
        xTo[:, :SHARD] = xb[pp.perms[c]].T.astype(np.float32)
        # column t*P+p must hold node at local position t*P+p:
        # perms[c] is already local-order, and xTo columns are local order.
        m["xTo"] = np.ascontiguousarray(xTo).astype(bf)
        m["idx"] = np.ascontiguousarray(pp.idx[c])
        m["mask"] = np.ascontiguousarray(pp.mask[c]).astype(bf)
        in_maps.append(m)
    return in_maps


def assemble_output(pp: Prep, results):
    out = np.zeros((N, CLS), np.float32)
    for c in range(NC):
        out[pp.perms[c]] = results[c]["y_out"][:SHARD]
    return out


_cache = {}
last_result = None


def kernel(**inputs) -> np.ndarray:
    global last_result
    trace = bool(int(os.environ.get("GAT_TRACE", "0")))
    if trace:
        _ensure_profile_hook()
    from concourse.bass_utils import run_bass_kernel_spmd

    ei = np.asarray(inputs["edge_index"])
    key = hash(ei.tobytes())
    if key not in _cache:
        pp = preprocess(ei)
        nc = build_program(pp)
        _cache[key] = (pp, nc)
    pp, nc = _cache[key]

    in_maps = make_in_maps(
        pp, inputs["x"], inputs["W0"], inputs["a_s0"], inputs["a_d0"],
        inputs["b0"], inputs["W1"], inputs["a_s1"], inputs["a_d1"],
        inputs["b1"], inputs["Wl"], inputs["bl"])
    res = run_bass_kernel_spmd(nc, in_maps, core_ids=list(range(NC)),
                               trace=trace)
    last_result = res
    return assemble_output(pp, res.results)


# revision 7
# speedup vs baseline: 1.7913x; 1.1669x over previous
"""Self-contained 8-core Trainium2 Bass kernel for a 2-layer GAT + linear classifier.

v2 design (dst-sharded 1D graph parallelism):
  - Host: add self-loops, degree-sort nodes, deal round-robin to 8 cores.
    Tiles of 128 dst nodes; uniform-degree groups of tiles (all tiles in a
    group share slot width D) so softmax reductions batch into single
    strided DVE instructions.  Pad slots handled by a -3e4 additive mask.
  - Layer 0 needs NO device gather and NO AllGather: the host pre-gathers
    x[src] per edge slot (bf16, transposed per 128-slot column) and the
    device computes [h|s|d] per slot with one PE matmul per column against
    a fused rhs [W0 | W0@a_s0 | W0@a_d0].
  - Layer 1: per-tile outputs [h1|s1|d1] = elu(agg0)@[W1|W1@a_s1|W1@a_d1]
    are written to a packed bf16 table [100000, 66], AllGathered (13 MB),
    then edge rows fetched with per-column indirect DMAs (the only
    HW-correct indexed-DMA form: one offset per partition per call).
  - Classifier fused per tile; one bulk DMA for the table and the output.
"""

import os
import sys
import types
from dataclasses import dataclass

import numpy as np
import ml_dtypes

P = 128
N = 100000
IN_DIM = 128
HID = 64
CLS = 40
NC = 8
HS = 66                      # table/slot row: [h(64) | s | d-or-junk]
SHARD = N // NC              # 12500
NT = (SHARD + P - 1) // P    # 98
NTP = NT * P                 # 12544
GS = 96                      # max slots per group
SLOPE = 0.2
MASKV = -30000.0


def _ensure_profile_hook():
    if "antenv.axon_hooks" in sys.modules:
        return
    try:
        import antenv
        mod = types.ModuleType("antenv.axon_hooks")
        mod._hook = None
        def _set(h):
            mod._hook = h
        def _get():
            return mod._hook
        mod.set_axon_ntff_profile_hook = _set
        mod.get_axon_ntff_profile_hook = _get
        sys.modules["antenv.axon_hooks"] = mod
        antenv.axon_hooks = mod
        from trn_agent_boot.trn_boot import _ntff_profile_via_ctypes
        _set(_ntff_profile_via_ctypes("/opt/axon/libaxon_pjrt.so"))
    except Exception:
        pass


# --------------------------------------------------------------------------
# Host preprocessing
# --------------------------------------------------------------------------

@dataclass
class Prep:
    perms: list          # per core: global node ids in local (row) order
    groups: list         # (t0, ntl, D, coff)
    C: int               # total slot columns
    srcs: list           # per core: [P, C] int64 src node per slot (pads=0)
    idx: list            # per core: [P, C] int32 table row per slot
    mask: list           # per core: [P, C] float32 0 / MASKV
    table_row: np.ndarray


def preprocess(edge_index) -> Prep:
    src = np.asarray(edge_index[0]).astype(np.int64)
    dst = np.asarray(edge_index[1]).astype(np.int64)
    loop = np.arange(N, dtype=np.int64)
    src_all = np.concatenate([src, loop])
    dst_all = np.concatenate([dst, loop])
    deg = np.bincount(dst_all, minlength=N).astype(np.int64)
    order = np.argsort(dst_all, kind="stable")
    srcs_by_dst = src_all[order]
    rowptr = np.zeros(N + 1, np.int64)
    np.cumsum(deg, out=rowptr[1:])

    rank_order = np.argsort(-deg, kind="stable")
    table_row = np.empty(N, np.int64)
    rr = np.arange(N)
    table_row[rank_order] = (rr % NC) * SHARD + rr // NC
    perms = [rank_order[c::NC] for c in range(NC)]

    rows_t = [min(P, SHARD - t * P) for t in range(NT)]
    D = np.zeros(NT, np.int64)
    for c in range(NC):
        dc = deg[perms[c]]
        for t in range(NT):
            D[t] = max(D[t], dc[t * P:t * P + rows_t[t]].max())

    groups = []
    t = 0
    coff = 0
    while t < NT:
        d = int(D[t])
        t0 = t
        t += 1
        while t < NT and int(D[t]) == d and (t - t0 + 1) * d <= GS:
            t += 1
        groups.append((t0, t - t0, d, coff))
        coff += (t - t0) * d
    C = coff

    srcs_l, idx_l, mask_l = [], [], []
    for c in range(NC):
        srcs = np.zeros((P, C), np.int64)
        mask = np.full((P, C), MASKV, np.float32)
        for (t0, ntl, d, goff) in groups:
            for ti in range(ntl):
                t = t0 + ti
                co = goff + ti * d
                rows = rows_t[t]
                nodes = perms[c][t * P:t * P + rows]
                degs = deg[nodes]
                starts = rowptr[nodes]
                pos = starts[:, None] + np.arange(d)[None, :]
                valid = np.arange(d)[None, :] < degs[:, None]
                blk = np.zeros((rows, d), np.int64)
                blk[valid] = srcs_by_dst[np.minimum(pos, rowptr[-1] - 1)[valid]]
                srcs[:rows, co:co + d] = blk
                m = np.full((rows, d), MASKV, np.float32)
                m[valid] = 0.0
                mask[:rows, co:co + d] = m
        srcs_l.append(srcs)
        idx_l.append(table_row[srcs].astype(np.int32))
        mask_l.append(mask)
    return Prep(perms=perms, groups=groups, C=C, srcs=srcs_l, idx=idx_l,
                mask=mask_l, table_row=table_row)


# --------------------------------------------------------------------------
# Device program
# --------------------------------------------------------------------------

NQ = 4                       # SWDGE queues for the indirect gathers


def indirect_gather_q(gp, out, in_, offset_ap, queue_name):
    """indirect_dma_start (src-indirect gather) pinned to a SWDGE queue."""
    import concourse.mybir as mybir
    from concourse.bass import BassSymbolicTensorAccessPattern

    src_ap = in_
    assert isinstance(src_ap.offset, int) and src_ap.offset == 0
    out_l = gp.lower_ap_dma(out, for_indirect_dma=True)
    in_l = gp.lower_ap_dma(in_, for_indirect_dma=True)
    assert len(in_l) == 1 and len(out_l) == 1
    off_l = gp.lower_ap_dma(offset_ap)
    assert len(off_l) == 1
    off_l = off_l[0]
    in_l.append(off_l)
    ap_shape = src_ap.shape
    coef = 1
    for i in range(1, len(ap_shape)):
        coef *= ap_shape[i]
    in_l[0].dynamic_ap_info = mybir.DynamicAccessPatternInfo(
        c=0,
        actual_ap=out.ap,
        indirect_dim_max_index=ap_shape[0],
        offset_expr=[
            mybir.DynamicAccessPatternOffsetExpr(
                coef=coef,
                aff_expr=mybir.DynamicAccessPatternOffsetExprAffExpr(
                    kind="IndirectArgId", arg_id=1),
            )
        ],
    )
    return gp.add_instruction(
        mybir.InstDMACopy(
            name=gp.bass.get_next_instruction_name(),
            queue=queue_name,
            mode="Copy",
            ins=in_l,
            outs=out_l,
            oob_is_err=True,
            cce_op=mybir.AluOpType.bypass,
        )
    )


def build_program(pp: Prep):
    import concourse.bass as bass
    import concourse.mybir as mybir
    import concourse.tile as tile
    from concourse import bacc

    f32 = mybir.dt.float32
    bf16 = mybir.dt.bfloat16
    i32 = mybir.dt.int32
    A = mybir.AluOpType
    AF = mybir.ActivationFunctionType
    C = pp.C
    NTLMAX = max(g[1] for g in pp.groups)

    nc = bacc.Bacc("TRN2", target_bir_lowering=False, debug=False,
                   num_devices=NC, num_swdge_queues=NQ)

    xgT_t = nc.dram_tensor("xgT", [P, C * P], bf16, kind="ExternalInput")
    xTo_t = nc.dram_tensor("xTo", [P, NTP], bf16, kind="ExternalInput")
    idx_t = nc.dram_tensor("idx", [P, C], i32, kind="ExternalInput")
    mask_t = nc.dram_tensor("mask", [P, C], bf16, kind="ExternalInput")
    rhs0_t = nc.dram_tensor("rhs0", [IN_DIM, HS], bf16, kind="ExternalInput")
    rhs1_t = nc.dram_tensor("rhs1", [HID, HS], bf16, kind="ExternalInput")
    rhsl_t = nc.dram_tensor("rhsl", [HID, CLS], bf16, kind="ExternalInput")
    b0r_t = nc.dram_tensor("b0r", [P, NTLMAX * HID], bf16, kind="ExternalInput")
    b1r_t = nc.dram_tensor("b1r", [P, NTLMAX * HID], bf16, kind="ExternalInput")
    blr_t = nc.dram_tensor("blr", [P, CLS], f32, kind="ExternalInput")
    id_t = nc.dram_tensor("ident", [P, P], bf16, kind="ExternalInput")
    y_t = nc.dram_tensor("y_out", [NTP, CLS], f32, kind="ExternalOutput")

    ha1_loc = nc.dram_tensor("ha1_loc", [NTP, HS], bf16, kind="Internal")
    ha1_full = nc.dram_tensor("ha1_full", [N, HS], bf16, kind="Internal",
                              addr_space="Shared")

    with tile.TileContext(nc) as tc:
        with tc.tile_pool(name="const", bufs=1) as cp, \
             tc.tile_pool(name="xgp", bufs=2) as xgp, \
             tc.tile_pool(name="gp", bufs=2) as gp, \
             tc.tile_pool(name="wp", bufs=3) as wp, \
             tc.tile_pool(name="pmm", bufs=3, space="PSUM") as pmm, \
             tc.tile_pool(name="ptr", bufs=2, space="PSUM") as ptr:

            def load_const(t, shape, dt):
                s = cp.tile(shape, dt, tag=f"c_{t.name}")
                nc.sync.dma_start(s[:], t.ap())
                return s

            rhs0_s = load_const(rhs0_t, [IN_DIM, HS], bf16)
            rhs1_s = load_const(rhs1_t, [HID, HS], bf16)
            rhsl_s = load_const(rhsl_t, [HID, CLS], bf16)
            b0r_s = load_const(b0r_t, [P, NTLMAX * HID], bf16)
            b1r_s = load_const(b1r_t, [P, NTLMAX * HID], bf16)
            blr_s = load_const(blr_t, [P, CLS], f32)
            ident_s = load_const(id_t, [P, P], bf16)
            idx_all = load_const(idx_t, [P, C], i32)
            mask_all = load_const(mask_t, [P, C], bf16)
            xTo_s = load_const(xTo_t, [P, NTP], bf16)

            ha1_sb = cp.tile([P, NT, HS], bf16)
            y_sb = cp.tile([P, NT, CLS], f32)
            d0_all = cp.tile([P, NT], f32)
            d1_all = cp.tile([P, NT], f32)

            # ---------------- phase A: d0 for own dst nodes ----------------
            for tb in range(0, NT, 7):
                tn = min(7, NT - tb)
                mm = pmm.tile([P, 7, HS], f32, space="PSUM", tag="mmG")
                for t2 in range(tn):
                    t = tb + t2
                    nc.tensor.matmul(out=mm[:, t2, :],
                                     lhsT=xTo_s[:, t * P:(t + 1) * P],
                                     rhs=rhs0_s[:], start=True, stop=True)
                nc.vector.tensor_copy(out=d0_all[:, tb:tb + tn],
                                      in_=mm[:, 0:tn, 65])

            # ---------------- edge phase (shared for both layers) ----------
            def edge_phase(layer, d_all, post):
                for (t0, ntl, D, goff) in pp.groups:
                    S = ntl * D
                    G = gp.tile([P, S, HS], bf16, tag="G")
                    if layer == 0:
                        xg = xgp.tile([P, S * P], bf16, tag="xg")
                        nc.sync.dma_start(
                            xg[:], xgT_t.ap()[:, goff * P:(goff + S) * P])
                        for jb in range(0, S, 7):
                            jn = min(7, S - jb)
                            mm = pmm.tile([P, 7, HS], f32, space="PSUM",
                                          tag="mmG")
                            for j2 in range(jn):
                                j = jb + j2
                                nc.tensor.matmul(
                                    out=mm[:, j2, :],
                                    lhsT=xg[:, j * P:(j + 1) * P],
                                    rhs=rhs0_s[:], start=True, stop=True)
                            nc.scalar.copy(G[:, jb:jb + jn, :],
                                           mm[:, 0:jn, :])
                    else:
                        for j in range(S):
                            q = (goff + j) % NQ
                            indirect_gather_q(
                                nc.gpsimd, G[:, j, :], ha1_full.ap(),
                                idx_all[:, goff + j:goff + j + 1],
                                f"qPoolDynamic{q or ''}")

                    # ---- segment softmax over each tile's D slots ----
                    dmap = wp.tile([P, S], bf16, tag="dmap")
                    for i in range(ntl):
                        nc.vector.tensor_copy(
                            out=dmap[:, i * D:(i + 1) * D],
                            in_=d_all[:, t0 + i:t0 + i + 1].to_broadcast(
                                [P, D]))
                    z = wp.tile([P, S], bf16, tag="z")
                    nc.vector.tensor_tensor(out=z[:], in0=G[:, :, 64],
                                            in1=dmap[:], op=A.add)
                    zs = wp.tile([P, S], bf16, tag="zs")
                    nc.vector.tensor_scalar(zs[:], z[:], SLOPE, None,
                                            op0=A.mult)
                    nc.vector.tensor_tensor(out=z[:], in0=z[:], in1=zs[:],
                                            op=A.max)
                    nc.vector.tensor_tensor(out=z[:], in0=z[:],
                                            in1=mask_all[:, goff:goff + S],
                                            op=A.add)
                    zv = z[:].rearrange("p (t d) -> p t d", t=ntl)
                    nm = wp.tile([P, ntl], bf16, tag="nm")
                    nc.vector.tensor_reduce(out=nm[:], in_=zv,
                                            axis=mybir.AxisListType.X,
                                            op=A.max, negate=True)
                    nc.vector.tensor_tensor(
                        out=zv, in0=zv,
                        in1=nm[:].to_broadcast([P, ntl, D]), op=A.add)
                    nc.scalar.activation(z[:], z[:], AF.Exp)
                    den = wp.tile([P, ntl], f32, tag="den")
                    nc.vector.tensor_reduce(out=den[:], in_=zv,
                                            axis=mybir.AxisListType.X,
                                            op=A.add)
                    rden = wp.tile([P, ntl], f32, tag="rden")
                    nc.vector.reciprocal(rden[:], den[:])
                    rb = wp.tile([P, ntl], bf16, tag="rb")
                    nc.vector.tensor_copy(out=rb[:], in_=rden[:])
                    nc.vector.tensor_tensor(
                        out=zv, in0=zv,
                        in1=rb[:].to_broadcast([P, ntl, D]), op=A.mult)
                    # ---- weighted aggregation: G *= alpha, tree-reduce ----
                    nc.vector.tensor_tensor(
                        out=G[:, :, 0:HID], in0=G[:, :, 0:HID],
                        in1=z[:].to_broadcast([P, S, HID]), op=A.mult)
                    G4 = G[:, :, :].rearrange("p (t d) e -> p t d e", t=ntl)
                    h = D
                    while h > 1:
                        a = (h + 1) // 2
                        nc.vector.tensor_tensor(
                            out=G4[:, :, 0:h - a, 0:HID],
                            in0=G4[:, :, 0:h - a, 0:HID],
                            in1=G4[:, :, a:h, 0:HID], op=A.add)
                        h = a
                    agg = G4[:, :, 0, 0:HID]          # [P, ntl, HID]
                    br = (b0r_s if layer == 0 else b1r_s)
                    hb = wp.tile([P, ntl * HID], bf16, tag="hb")
                    nc.vector.tensor_tensor(
                        out=hb[:].rearrange("p (t e) -> p t e", t=ntl),
                        in0=agg,
                        in1=br[:, 0:ntl * HID].rearrange(
                            "p (t e) -> p t e", t=ntl), op=A.add)
                    ex = wp.tile([P, ntl * HID], bf16, tag="ex")
                    nc.scalar.activation(ex[:], hb[:], AF.Exp)
                    nc.vector.tensor_scalar(ex[:], ex[:], -1.0, 0.0,
                                            op0=A.add, op1=A.min)
                    rl = wp.tile([P, ntl * HID], bf16, tag="rl")
                    nc.vector.tensor_scalar(rl[:], hb[:], 0.0, None,
                                            op0=A.max)
                    ht = wp.tile([P, ntl * HID], bf16, tag="ht")
                    nc.vector.tensor_tensor(out=ht[:], in0=rl[:], in1=ex[:],
                                            op=A.add)
                    post(t0, ntl, ht)

            # ---------------- posts ----------------
            def transpose_pairs(ntl, ht, consume):
                for i in range(ntl):
                    tp = ptr.tile([HID, P], bf16, space="PSUM", tag="tp")
                    nc.tensor.transpose(out=tp[:, :],
                                        in_=ht[:, i * HID:(i + 1) * HID],
                                        identity=ident_s[:])
                    hT2 = wp.tile([HID, P], bf16, tag="hT2")
                    nc.scalar.copy(hT2[:, :], tp[:, :])
                    consume(i, hT2[:, :])

            def post_l0(t0, ntl, ht):
                def consume(i2, lhsT):
                    t = t0 + i2
                    mm = pmm.tile([P, HS], f32, space="PSUM", tag="mmP")
                    nc.tensor.matmul(out=mm[:, :], lhsT=lhsT, rhs=rhs1_s[:],
                                     start=True, stop=True)
                    nc.scalar.copy(ha1_sb[:, t, :], mm[:, :])
                    nc.vector.tensor_copy(out=d1_all[:, t:t + 1],
                                          in_=mm[:, 65:66])
                transpose_pairs(ntl, ht, consume)

            def post_l1(t0, ntl, ht):
                def consume(i2, lhsT):
                    t = t0 + i2
                    mm = pmm.tile([P, HS], f32, space="PSUM", tag="mmP")
                    nc.tensor.matmul(out=mm[:, 0:CLS], lhsT=lhsT, rhs=rhsl_s[:],
                                     start=True, stop=True)
                    nc.vector.tensor_tensor(out=y_sb[:, t, :], in0=mm[:, 0:CLS],
                                            in1=blr_s[:], op=A.add)
                transpose_pairs(ntl, ht, consume)

            # ---------------- run ----------------
            edge_phase(0, d0_all, post_l0)
            nc.sync.dma_start(
                ha1_loc.ap().rearrange("(t p) e -> p t e", p=P),
                ha1_sb[:, :, :])
            nc.gpsimd.collective_compute(
                "AllGather", A.bypass,
                replica_groups=[list(range(NC))],
                ins=[ha1_loc.ap()[0:SHARD, :]],
                outs=[ha1_full.ap()[0:N, :]],
            )
            edge_phase(1, d1_all, post_l1)
            nc.sync.dma_start(
                y_t.ap().rearrange("(t p) e -> p t e", p=P),
                y_sb[:, :, :])

    nc.compile()
    return nc


# --------------------------------------------------------------------------
# Input staging / output assembly
# --------------------------------------------------------------------------

def make_in_maps(pp: Prep, x, W0, a_s0, a_d0, b0, W1, a_s1, a_d1, b1, Wl, bl):
    bf = ml_dtypes.bfloat16
    x = np.asarray(x, np.float32)
    W0 = np.asarray(W0, np.float32)
    W1 = np.asarray(W1, np.float32)
    Wl = np.asarray(Wl, np.float32)
    NTLMAX = max(g[1] for g in pp.groups)

    rhs0 = np.concatenate(
        [W0, (W0 @ np.asarray(a_s0, np.float32))[:, None],
         (W0 @ np.asarray(a_d0, np.float32))[:, None]], axis=1)
    rhs1 = np.concatenate(
        [W1, (W1 @ np.asarray(a_s1, np.float32))[:, None],
         (W1 @ np.asarray(a_d1, np.float32))[:, None]], axis=1)
    consts = dict(
        rhs0=np.ascontiguousarray(rhs0).astype(bf),
        rhs1=np.ascontiguousarray(rhs1).astype(bf),
        rhsl=np.ascontiguousarray(Wl).astype(bf),
        b0r=np.ascontiguousarray(
            np.tile(np.asarray(b0, np.float32)[None, :], (P, NTLMAX))).astype(bf),
        b1r=np.ascontiguousarray(
            np.tile(np.asarray(b1, np.float32)[None, :], (P, NTLMAX))).astype(bf),
        blr=np.ascontiguousarray(
            np.tile(np.asarray(bl, np.float32)[None, :], (P, 1))),
        ident=np.eye(P, dtype=np.float32).astype(bf),
    )
    xb = x.astype(bf)
    in_maps = []
    for c in range(NC):
        m = dict(consts)
        xg = xb[pp.srcs[c]]                       # [P, C, IN_DIM]
        m["xgT"] = np.ascontiguousarray(
            xg.transpose(2, 1, 0).reshape(IN_DIM, pp.C * P))
        xTo = np.zeros((P, NTP), np.float32)
        xTo[:, :SHARD] = xb[pp.perms[c]].T.astype(np.float32)
        # column t*P+p must hold node at local position t*P+p:
        # perms[c] is already local-order, and xTo columns are local order.
        m["xTo"] = np.ascontiguousarray(xTo).astype(bf)
        m["idx"] = np.ascontiguousarray(pp.idx[c])
        m["mask"] = np.ascontiguousarray(pp.mask[c]).astype(bf)
        in_maps.append(m)
    return in_maps


def assemble_output(pp: Prep, results):
    out = np.zeros((N, CLS), np.float32)
    for c in range(NC):
        out[pp.perms[c]] = results[c]["y_out"][:SHARD]
    return out


_cache = {}
last_result = None


def kernel(**inputs) -> np.ndarray:
    global last_result
    trace = bool(int(os.environ.get("GAT_TRACE", "0")))
    if trace:
        _ensure_profile_hook()
    from concourse.bass_utils import run_bass_kernel_spmd

    ei = np.asarray(inputs["edge_index"])
    key = hash(ei.tobytes())
    if key not in _cache:
        pp = preprocess(ei)
        nc = build_program(pp)
        _cache[key] = (pp, nc)
    pp, nc = _cache[key]

    in_maps = make_in_maps(
        pp, inputs["x"], inputs["W0"], inputs["a_s0"], inputs["a_d0"],
        inputs["b0"], inputs["W1"], inputs["a_s1"], inputs["a_d1"],
        inputs["b1"], inputs["Wl"], inputs["bl"])
    res = run_bass_kernel_spmd(nc, in_maps, core_ids=list(range(NC)),
                               trace=trace)
    last_result = res
    return assemble_output(pp, res.results)


# revision 12
# speedup vs baseline: 1.7970x; 1.0032x over previous
"""Self-contained 8-core Trainium2 Bass kernel for a 2-layer GAT + linear classifier.

v2 design (dst-sharded 1D graph parallelism):
  - Host: add self-loops, degree-sort nodes, deal round-robin to 8 cores.
    Tiles of 128 dst nodes; uniform-degree groups of tiles (all tiles in a
    group share slot width D) so softmax reductions batch into single
    strided DVE instructions.  Pad slots handled by a -3e4 additive mask.
  - Layer 0 needs NO device gather and NO AllGather: the host pre-gathers
    x[src] per edge slot (bf16, transposed per 128-slot column) and the
    device computes [h|s|d] per slot with one PE matmul per column against
    a fused rhs [W0 | W0@a_s0 | W0@a_d0].
  - Layer 1: per-tile outputs [h1|s1|d1] = elu(agg0)@[W1|W1@a_s1|W1@a_d1]
    are written to a packed bf16 table [100000, 66], AllGathered (13 MB),
    then edge rows fetched with per-column indirect DMAs (the only
    HW-correct indexed-DMA form: one offset per partition per call).
  - Classifier fused per tile; one bulk DMA for the table and the output.
"""

import os
import sys
import types
from dataclasses import dataclass

import numpy as np
import ml_dtypes

P = 128
N = 100000
IN_DIM = 128
HID = 64
CLS = 40
NC = 8
HS = 66                      # table/slot row: [h(64) | s | d-or-junk]
SHARD = N // NC              # 12500
NT = (SHARD + P - 1) // P    # 98
NTP = NT * P                 # 12544
GS = 96                      # max slots per group
SLOPE = 0.2
MASKV = -30000.0


def _ensure_profile_hook():
    if "antenv.axon_hooks" in sys.modules:
        return
    try:
        import antenv
        mod = types.ModuleType("antenv.axon_hooks")
        mod._hook = None
        def _set(h):
            mod._hook = h
        def _get():
            return mod._hook
        mod.set_axon_ntff_profile_hook = _set
        mod.get_axon_ntff_profile_hook = _get
        sys.modules["antenv.axon_hooks"] = mod
        antenv.axon_hooks = mod
        from trn_agent_boot.trn_boot import _ntff_profile_via_ctypes
        _set(_ntff_profile_via_ctypes("/opt/axon/libaxon_pjrt.so"))
    except Exception:
        pass


# --------------------------------------------------------------------------
# Host preprocessing
# --------------------------------------------------------------------------

@dataclass
class Prep:
    perms: list          # per core: global node ids in local (row) order
    groups: list         # (t0, ntl, D, coff)
    C: int               # total slot columns
    srcs: list           # per core: [P, C] int64 src node per slot (pads=0)
    idx: list            # per core: [P, C] int32 table row per slot
    mask: list           # per core: [P, C] float32 0 / MASKV
    table_row: np.ndarray


def preprocess(edge_index) -> Prep:
    # Explicit self-loops occupy slot column 0 of every tile (served from
    # local SBUF on device, no gather); only the raw edges get slots 1..deg.
    src_all = np.asarray(edge_index[0]).astype(np.int64)
    dst_all = np.asarray(edge_index[1]).astype(np.int64)
    deg_ns = np.bincount(dst_all, minlength=N).astype(np.int64)
    deg = deg_ns + 1
    order = np.argsort(dst_all, kind="stable")
    srcs_by_dst = src_all[order]
    rowptr = np.zeros(N + 1, np.int64)
    np.cumsum(deg_ns, out=rowptr[1:])

    rank_order = np.argsort(-deg, kind="stable")
    table_row = np.empty(N, np.int64)
    rr = np.arange(N)
    table_row[rank_order] = (rr % NC) * SHARD + rr // NC
    perms = [rank_order[c::NC] for c in range(NC)]

    rows_t = [min(P, SHARD - t * P) for t in range(NT)]
    D = np.zeros(NT, np.int64)
    for c in range(NC):
        dc = deg[perms[c]]
        for t in range(NT):
            D[t] = max(D[t], dc[t * P:t * P + rows_t[t]].max())

    groups = []
    t = 0
    coff = 0
    while t < NT:
        d = int(D[t])
        t0 = t
        t += 1
        while t < NT and int(D[t]) == d and (t - t0 + 1) * d <= GS:
            t += 1
        groups.append((t0, t - t0, d, coff))
        coff += (t - t0) * d
    C = coff

    srcs_l, idx_l, mask_l = [], [], []
    for c in range(NC):
        srcs = np.zeros((P, C), np.int64)
        mask = np.full((P, C), MASKV, np.float32)
        for (t0, ntl, d, goff) in groups:
            for ti in range(ntl):
                t = t0 + ti
                co = goff + ti * d
                rows = rows_t[t]
                nodes = perms[c][t * P:t * P + rows]
                # column 0: self-loop
                srcs[:rows, co] = nodes
                mask[:rows, co] = 0.0
                # columns 1..d-1: raw edges
                dn = d - 1
                degs = deg_ns[nodes]
                starts = rowptr[nodes]
                pos = starts[:, None] + np.arange(dn)[None, :]
                valid = np.arange(dn)[None, :] < degs[:, None]
                blk = np.zeros((rows, dn), np.int64)
                blk[valid] = srcs_by_dst[np.minimum(pos, rowptr[-1] - 1)[valid]]
                srcs[:rows, co + 1:co + d] = blk
                m = np.full((rows, dn), MASKV, np.float32)
                m[valid] = 0.0
                mask[:rows, co + 1:co + d] = m
        srcs_l.append(srcs)
        idx_l.append(table_row[srcs].astype(np.int32))
        mask_l.append(mask)
    return Prep(perms=perms, groups=groups, C=C, srcs=srcs_l, idx=idx_l,
                mask=mask_l, table_row=table_row)


# --------------------------------------------------------------------------
# Device program
# --------------------------------------------------------------------------

NQ = 4                       # SWDGE queues for the indirect gathers


def indirect_gather_q(gp, out, in_, offset_ap, queue_name):
    """indirect_dma_start (src-indirect gather) pinned to a SWDGE queue."""
    import concourse.mybir as mybir
    from concourse.bass import BassSymbolicTensorAccessPattern

    src_ap = in_
    assert isinstance(src_ap.offset, int) and src_ap.offset == 0
    out_l = gp.lower_ap_dma(out, for_indirect_dma=True)
    in_l = gp.lower_ap_dma(in_, for_indirect_dma=True)
    assert len(in_l) == 1 and len(out_l) == 1
    off_l = gp.lower_ap_dma(offset_ap)
    assert len(off_l) == 1
    off_l = off_l[0]
    in_l.append(off_l)
    ap_shape = src_ap.shape
    coef = 1
    for i in range(1, len(ap_shape)):
        coef *= ap_shape[i]
    in_l[0].dynamic_ap_info = mybir.DynamicAccessPatternInfo(
        c=0,
        actual_ap=out.ap,
        indirect_dim_max_index=ap_shape[0],
        offset_expr=[
            mybir.DynamicAccessPatternOffsetExpr(
                coef=coef,
                aff_expr=mybir.DynamicAccessPatternOffsetExprAffExpr(
                    kind="IndirectArgId", arg_id=1),
            )
        ],
    )
    return gp.add_instruction(
        mybir.InstDMACopy(
            name=gp.bass.get_next_instruction_name(),
            queue=queue_name,
            mode="Copy",
            ins=in_l,
            outs=out_l,
            oob_is_err=True,
            cce_op=mybir.AluOpType.bypass,
        )
    )


def build_program(pp: Prep):
    import concourse.bass as bass
    import concourse.mybir as mybir
    import concourse.tile as tile
    from concourse import bacc

    f32 = mybir.dt.float32
    bf16 = mybir.dt.bfloat16
    i32 = mybir.dt.int32
    A = mybir.AluOpType
    AF = mybir.ActivationFunctionType
    C = pp.C
    NTLMAX = max(g[1] for g in pp.groups)

    nc = bacc.Bacc("TRN2", target_bir_lowering=False, debug=False,
                   num_devices=NC, num_swdge_queues=NQ)

    xgT_t = nc.dram_tensor("xgT", [P, C * P], bf16, kind="ExternalInput")
    xTo_t = nc.dram_tensor("xTo", [P, NTP], bf16, kind="ExternalInput")
    idx_t = nc.dram_tensor("idx", [P, C], i32, kind="ExternalInput")
    mask_t = nc.dram_tensor("mask", [P, C], bf16, kind="ExternalInput")
    rhs0_t = nc.dram_tensor("rhs0", [IN_DIM, HS], bf16, kind="ExternalInput")
    rhs1_t = nc.dram_tensor("rhs1", [HID, HS], bf16, kind="ExternalInput")
    rhsl_t = nc.dram_tensor("rhsl", [HID, CLS], bf16, kind="ExternalInput")
    b0r_t = nc.dram_tensor("b0r", [P, NTLMAX * HID], bf16, kind="ExternalInput")
    b1r_t = nc.dram_tensor("b1r", [P, NTLMAX * HID], bf16, kind="ExternalInput")
    blr_t = nc.dram_tensor("blr", [P, CLS], f32, kind="ExternalInput")
    id_t = nc.dram_tensor("ident", [P, P], bf16, kind="ExternalInput")
    y_t = nc.dram_tensor("y_out", [NTP, CLS], f32, kind="ExternalOutput")

    ha1_loc = nc.dram_tensor("ha1_loc", [NTP, HS], bf16, kind="Internal")
    ha1_full = nc.dram_tensor("ha1_full", [N, HS], bf16, kind="Internal",
                              addr_space="Shared")

    with tile.TileContext(nc) as tc:
        with tc.tile_pool(name="const", bufs=1) as cp, \
             tc.tile_pool(name="xgp", bufs=2) as xgp, \
             tc.tile_pool(name="gp", bufs=2) as gp, \
             tc.tile_pool(name="wp", bufs=3) as wp, \
             tc.tile_pool(name="pmm", bufs=3, space="PSUM") as pmm, \
             tc.tile_pool(name="ptr", bufs=2, space="PSUM") as ptr:

            def load_const(t, shape, dt):
                s = cp.tile(shape, dt, tag=f"c_{t.name}")
                nc.sync.dma_start(s[:], t.ap())
                return s

            rhs0_s = load_const(rhs0_t, [IN_DIM, HS], bf16)
            rhs1_s = load_const(rhs1_t, [HID, HS], bf16)
            rhsl_s = load_const(rhsl_t, [HID, CLS], bf16)
            b0r_s = load_const(b0r_t, [P, NTLMAX * HID], bf16)
            b1r_s = load_const(b1r_t, [P, NTLMAX * HID], bf16)
            blr_s = load_const(blr_t, [P, CLS], f32)
            ident_s = load_const(id_t, [P, P], bf16)
            idx_all = load_const(idx_t, [P, C], i32)
            mask_all = load_const(mask_t, [P, C], bf16)
            xTo_s = load_const(xTo_t, [P, NTP], bf16)

            ha1_sb = cp.tile([P, NT, HS], bf16)
            y_sb = cp.tile([P, NT, CLS], f32)
            d0_all = cp.tile([P, NT], bf16)
            d1_all = cp.tile([P, NT], bf16)
            dmask0 = cp.tile([P, C], bf16)
            dmask1 = cp.tile([P, C], bf16)

            # ---------------- phase A: d0 for own dst nodes ----------------
            for tb in range(0, NT, 7):
                tn = min(7, NT - tb)
                mm = pmm.tile([P, 7, HS], f32, space="PSUM", tag="mmG")
                for t2 in range(tn):
                    t = tb + t2
                    nc.tensor.matmul(out=mm[:, t2, :],
                                     lhsT=xTo_s[:, t * P:(t + 1) * P],
                                     rhs=rhs0_s[:], start=True, stop=True)
                nc.vector.tensor_copy(out=d0_all[:, tb:tb + tn],
                                      in_=mm[:, 0:tn, 65])

            # ---------------- edge phase (shared for both layers) ----------
            def build_dmask(dmask, d_all):
                for (t0, ntl, D, goff) in pp.groups:
                    S = ntl * D
                    nc.vector.tensor_tensor(
                        out=dmask[:, goff:goff + S].rearrange(
                            "p (t d) -> p t d", t=ntl),
                        in0=mask_all[:, goff:goff + S].rearrange(
                            "p (t d) -> p t d", t=ntl),
                        in1=d_all[:, t0:t0 + ntl].to_broadcast([P, ntl, D]),
                        op=A.add)

            def edge_phase(layer, dmask, post):
                for (t0, ntl, D, goff) in pp.groups:
                    S = ntl * D
                    G = gp.tile([P, S, HS], bf16, tag="G")
                    if layer == 0:
                        xg = xgp.tile([P, S * P], bf16, tag="xg")
                        nc.sync.dma_start(
                            xg[:], xgT_t.ap()[:, goff * P:(goff + S) * P])
                        for jb in range(0, S, 7):
                            jn = min(7, S - jb)
                            mm = pmm.tile([P, 7, HS], f32, space="PSUM",
                                          tag="mmG")
                            for j2 in range(jn):
                                j = jb + j2
                                nc.tensor.matmul(
                                    out=mm[:, j2, :],
                                    lhsT=xg[:, j * P:(j + 1) * P],
                                    rhs=rhs0_s[:], start=True, stop=True)
                            nc.scalar.copy(G[:, jb:jb + jn, :],
                                           mm[:, 0:jn, :])
                    else:
                        for j in range(S):
                            if j % D == 0:       # self-loop column: local copy
                                nc.scalar.copy(G[:, j, :],
                                               ha1_sb[:, t0 + j // D, :])
                                continue
                            q = (goff + j) % NQ
                            indirect_gather_q(
                                nc.gpsimd, G[:, j, :], ha1_full.ap(),
                                idx_all[:, goff + j:goff + j + 1],
                                f"qPoolDynamic{q or ''}")

                    # ---- segment softmax over each tile's D slots ----
                    z = wp.tile([P, S], bf16, tag="z")
                    nc.vector.tensor_tensor(out=z[:], in0=G[:, :, 64],
                                            in1=dmask[:, goff:goff + S],
                                            op=A.add)
                    zs = wp.tile([P, S], bf16, tag="zs")
                    nc.vector.tensor_scalar(zs[:], z[:], SLOPE, None,
                                            op0=A.mult)
                    nc.vector.tensor_tensor(out=z[:], in0=z[:], in1=zs[:],
                                            op=A.max)
                    zv = z[:].rearrange("p (t d) -> p t d", t=ntl)
                    nm = wp.tile([P, ntl], bf16, tag="nm")
                    nc.vector.tensor_reduce(out=nm[:], in_=zv,
                                            axis=mybir.AxisListType.X,
                                            op=A.max, negate=True)
                    den = wp.tile([P, ntl], f32, tag="den")
                    for i in range(ntl):
                        nc.scalar.activation(
                            z[:, i * D:(i + 1) * D], z[:, i * D:(i + 1) * D],
                            AF.Exp, bias=nm[:, i:i + 1], scale=1.0,
                            accum_out=den[:, i:i + 1])
                    rden = wp.tile([P, ntl], f32, tag="rden")
                    nc.vector.reciprocal(rden[:], den[:])
                    rb = wp.tile([P, ntl], bf16, tag="rb")
                    nc.vector.tensor_copy(out=rb[:], in_=rden[:])
                    nc.vector.tensor_tensor(
                        out=zv, in0=zv,
                        in1=rb[:].to_broadcast([P, ntl, D]), op=A.mult)
                    # ---- weighted aggregation: G *= alpha, tree-reduce ----
                    amul = nc.gpsimd if layer == 0 else nc.vector
                    amul.tensor_tensor(
                        out=G[:, :, 0:HID], in0=G[:, :, 0:HID],
                        in1=z[:].to_broadcast([P, S, HID]), op=A.mult)
                    G4 = G[:, :, :].rearrange("p (t d) e -> p t d e", t=ntl)
                    h = D
                    while h > 1:
                        a = (h + 1) // 2
                        nc.vector.tensor_tensor(
                            out=G4[:, :, 0:h - a, 0:HID],
                            in0=G4[:, :, 0:h - a, 0:HID],
                            in1=G4[:, :, a:h, 0:HID], op=A.add)
                        h = a
                    agg = G4[:, :, 0, 0:HID]          # [P, ntl, HID]
                    br = (b0r_s if layer == 0 else b1r_s)
                    hb = wp.tile([P, ntl * HID], bf16, tag="hb")
                    nc.vector.tensor_tensor(
                        out=hb[:].rearrange("p (t e) -> p t e", t=ntl),
                        in0=agg,
                        in1=br[:, 0:ntl * HID].rearrange(
                            "p (t e) -> p t e", t=ntl), op=A.add)
                    ex = wp.tile([P, ntl * HID], bf16, tag="ex")
                    nc.scalar.activation(ex[:], hb[:], AF.Exp)
                    nc.vector.tensor_scalar(ex[:], ex[:], -1.0, 0.0,
                                            op0=A.add, op1=A.min)
                    rl = wp.tile([P, ntl * HID], bf16, tag="rl")
                    nc.vector.tensor_scalar(rl[:], hb[:], 0.0, None,
                                            op0=A.max)
                    ht = wp.tile([P, ntl * HID], bf16, tag="ht")
                    nc.vector.tensor_tensor(out=ht[:], in0=rl[:], in1=ex[:],
                                            op=A.add)
                    post(t0, ntl, ht)

            # ---------------- posts ----------------
            def transpose_pairs(ntl, ht, consume):
                for i in range(ntl):
                    tp = ptr.tile([HID, P], bf16, space="PSUM", tag="tp")
                    nc.tensor.transpose(out=tp[:, :],
                                        in_=ht[:, i * HID:(i + 1) * HID],
                                        identity=ident_s[:])
                    hT2 = wp.tile([HID, P], bf16, tag="hT2")
                    nc.scalar.copy(hT2[:, :], tp[:, :])
                    consume(i, hT2[:, :])

            def post_l0(t0, ntl, ht):
                def consume(i2, lhsT):
                    t = t0 + i2
                    mm = pmm.tile([P, HS], f32, space="PSUM", tag="mmP")
                    nc.tensor.matmul(out=mm[:, :], lhsT=lhsT, rhs=rhs1_s[:],
                                     start=True, stop=True)
                    nc.scalar.copy(ha1_sb[:, t, :], mm[:, :])
                    nc.vector.tensor_copy(out=d1_all[:, t:t + 1],
                                          in_=mm[:, 65:66])
                transpose_pairs(ntl, ht, consume)

            def post_l1(t0, ntl, ht):
                def consume(i2, lhsT):
                    t = t0 + i2
                    mm = pmm.tile([P, HS], f32, space="PSUM", tag="mmP")
                    nc.tensor.matmul(out=mm[:, 0:CLS], lhsT=lhsT, rhs=rhsl_s[:],
                                     start=True, stop=True)
                    nc.vector.tensor_tensor(out=y_sb[:, t, :], in0=mm[:, 0:CLS],
                                            in1=blr_s[:], op=A.add)
                transpose_pairs(ntl, ht, consume)

            # ---------------- run ----------------
            build_dmask(dmask0, d0_all)
            edge_phase(0, dmask0, post_l0)
            nc.sync.dma_start(
                ha1_loc.ap().rearrange("(t p) e -> p t e", p=P),
                ha1_sb[:, :, :])
            nc.gpsimd.collective_compute(
                "AllGather", A.bypass,
                replica_groups=[list(range(NC))],
                ins=[ha1_loc.ap()[0:SHARD, :]],
                outs=[ha1_full.ap()[0:N, :]],
            )
            build_dmask(dmask1, d1_all)
            edge_phase(1, dmask1, post_l1)
            nc.sync.dma_start(
                y_t.ap().rearrange("(t p) e -> p t e", p=P),
                y_sb[:, :, :])

    nc.compile()
    return nc


# --------------------------------------------------------------------------
# Input staging / output assembly
# --------------------------------------------------------------------------

def make_in_maps(pp: Prep, x, W0, a_s0, a_d0, b0, W1, a_s1, a_d1, b1, Wl, bl):
    bf = ml_dtypes.bfloat16
    x = np.asarray(x, np.float32)
    W0 = np.asarray(W0, np.float32)
    W1 = np.asarray(W1, np.float32)
    Wl = np.asarray(Wl, np.float32)
    NTLMAX = max(g[1] for g in pp.groups)

    rhs0 = np.concatenate(
        [W0, (W0 @ np.asarray(a_s0, np.float32))[:, None],
         (W0 @ np.asarray(a_d0, np.float32))[:, None]], axis=1)
    rhs1 = np.concatenate(
        [W1, (W1 @ np.asarray(a_s1, np.float32))[:, None],
         (W1 @ np.asarray(a_d1, np.float32))[:, None]], axis=1)
    consts = dict(
        rhs0=np.ascontiguousarray(rhs0).astype(bf),
        rhs1=np.ascontiguousarray(rhs1).astype(bf),
        rhsl=np.ascontiguousarray(Wl).astype(bf),
        b0r=np.ascontiguousarray(
            np.tile(np.asarray(b0, np.float32)[None, :], (P, NTLMAX))).astype(bf),
        b1r=np.ascontiguousarray(
            np.tile(np.asarray(b1, np.float32)[None, :], (P, NTLMAX))).astype(bf),
        blr=np.ascontiguousarray(
            np.tile(np.asarray(bl, np.float32)[None, :], (P, 1))),
        ident=np.eye(P, dtype=np.float32).astype(bf),
    )
    xb = x.astype(bf)
    in_maps = []
    for c in range(NC):
        m = dict(consts)
        xg = xb[pp.srcs[c]]                       # [P, C, IN_DIM]
        m["xgT"] = np.ascontiguousarray(
            xg.transpose(2, 1, 0).reshape(IN_DIM, pp.C * P))
        xTo = np.zeros((P, NTP), np.float32)
        xTo[:, :SHARD] = xb[pp.perms[c]].T.astype(np.float32)
        # column t*P+p must hold node at local position t*P+p:
        # perms[c] is already local-order, and xTo columns are local order.
        m["xTo"] = np.ascontiguousarray(xTo).astype(bf)
        m["idx"] = np.ascontiguousarray(pp.idx[c])
        m["mask"] = np.ascontiguousarray(pp.mask[c]).astype(bf)
        in_maps.append(m)
    return in_maps


def assemble_output(pp: Prep, results):
    out = np.zeros((N, CLS), np.float32)
    for c in range(NC):
        out[pp.perms[c]] = results[c]["y_out"][:SHARD]
    return out


_cache = {}
last_result = None


def kernel(**inputs) -> np.ndarray:
    global last_result
    trace = bool(int(os.environ.get("GAT_TRACE", "0")))
    if trace:
        _ensure_profile_hook()
    from concourse.bass_utils import run_bass_kernel_spmd

    ei = np.asarray(inputs["edge_index"])
    key = hash(ei.tobytes())
    if key not in _cache:
        pp = preprocess(ei)
        nc = build_program(pp)
        _cache[key] = (pp, nc)
    pp, nc = _cache[key]

    in_maps = make_in_maps(
        pp, inputs["x"], inputs["W0"], inputs["a_s0"], inputs["a_d0"],
        inputs["b0"], inputs["W1"], inputs["a_s1"], inputs["a_d1"],
        inputs["b1"], inputs["Wl"], inputs["bl"])
    res = run_bass_kernel_spmd(nc, in_maps, core_ids=list(range(NC)),
                               trace=trace)
    last_result = res
    return assemble_output(pp, res.results)


# revision 13
# speedup vs baseline: 1.8606x; 1.0354x over previous
"""Self-contained 8-core Trainium2 Bass kernel for a 2-layer GAT + linear classifier.

v2 design (dst-sharded 1D graph parallelism):
  - Host: add self-loops, degree-sort nodes, deal round-robin to 8 cores.
    Tiles of 128 dst nodes; uniform-degree groups of tiles (all tiles in a
    group share slot width D) so softmax reductions batch into single
    strided DVE instructions.  Pad slots handled by a -3e4 additive mask.
  - Layer 0 needs NO device gather and NO AllGather: the host pre-gathers
    x[src] per edge slot (bf16, transposed per 128-slot column) and the
    device computes [h|s|d] per slot with one PE matmul per column against
    a fused rhs [W0 | W0@a_s0 | W0@a_d0].
  - Layer 1: per-tile outputs [h1|s1|d1] = elu(agg0)@[W1|W1@a_s1|W1@a_d1]
    are written to a packed bf16 table [100000, 66], AllGathered (13 MB),
    then edge rows fetched with per-column indirect DMAs (the only
    HW-correct indexed-DMA form: one offset per partition per call).
  - Classifier fused per tile; one bulk DMA for the table and the output.
"""

import os
import sys
import types
from dataclasses import dataclass

import numpy as np
import ml_dtypes

P = 128
N = 100000
IN_DIM = 128
HID = 64
CLS = 40
NC = 8
HS = 66                      # table/slot row: [h(64) | s | d-or-junk]
SHARD = N // NC              # 12500
NT = (SHARD + P - 1) // P    # 98
NTP = NT * P                 # 12544
GS = 96                      # max slots per group
SLOPE = 0.2
MASKV = -30000.0


def _ensure_profile_hook():
    if "antenv.axon_hooks" in sys.modules:
        return
    try:
        import antenv
        mod = types.ModuleType("antenv.axon_hooks")
        mod._hook = None
        def _set(h):
            mod._hook = h
        def _get():
            return mod._hook
        mod.set_axon_ntff_profile_hook = _set
        mod.get_axon_ntff_profile_hook = _get
        sys.modules["antenv.axon_hooks"] = mod
        antenv.axon_hooks = mod
        from trn_agent_boot.trn_boot import _ntff_profile_via_ctypes
        _set(_ntff_profile_via_ctypes("/opt/axon/libaxon_pjrt.so"))
    except Exception:
        pass


# --------------------------------------------------------------------------
# Host preprocessing
# --------------------------------------------------------------------------

@dataclass
class Prep:
    perms: list          # per core: global node ids in local (row) order
    groups: list         # (t0, ntl, D, coff)
    C: int               # total slot columns
    srcs: list           # per core: [P, C] int64 src node per slot (pads=0)
    idx: list            # per core: [P, C] int32 table row per slot
    mask: list           # per core: [P, C] float32 0 / MASKV
    table_row: np.ndarray


def preprocess(edge_index) -> Prep:
    # Explicit self-loops occupy slot column 0 of every tile (served from
    # local SBUF on device, no gather); only the raw edges get slots 1..deg.
    src_all = np.asarray(edge_index[0]).astype(np.int64)
    dst_all = np.asarray(edge_index[1]).astype(np.int64)
    deg_ns = np.bincount(dst_all, minlength=N).astype(np.int64)
    deg = deg_ns + 1
    order = np.argsort(dst_all, kind="stable")
    srcs_by_dst = src_all[order]
    rowptr = np.zeros(N + 1, np.int64)
    np.cumsum(deg_ns, out=rowptr[1:])

    rank_order = np.argsort(-deg, kind="stable")
    table_row = np.empty(N, np.int64)
    rr = np.arange(N)
    table_row[rank_order] = (rr % NC) * SHARD + rr // NC
    perms = [rank_order[c::NC] for c in range(NC)]

    rows_t = [min(P, SHARD - t * P) for t in range(NT)]
    D = np.zeros(NT, np.int64)
    for c in range(NC):
        dc = deg[perms[c]]
        for t in range(NT):
            D[t] = max(D[t], dc[t * P:t * P + rows_t[t]].max())

    groups = []
    t = 0
    coff = 0
    while t < NT:
        d = int(D[t])
        t0 = t
        t += 1
        while t < NT and int(D[t]) == d and (t - t0 + 1) * d <= GS:
            t += 1
        groups.append((t0, t - t0, d, coff))
        coff += (t - t0) * d
    C = coff

    srcs_l, idx_l, mask_l = [], [], []
    for c in range(NC):
        srcs = np.zeros((P, C), np.int64)
        mask = np.full((P, C), MASKV, np.float32)
        for (t0, ntl, d, goff) in groups:
            for ti in range(ntl):
                t = t0 + ti
                co = goff + ti * d
                rows = rows_t[t]
                nodes = perms[c][t * P:t * P + rows]
                # column 0: self-loop
                srcs[:rows, co] = nodes
                mask[:rows, co] = 0.0
                # columns 1..d-1: raw edges
                dn = d - 1
                degs = deg_ns[nodes]
                starts = rowptr[nodes]
                pos = starts[:, None] + np.arange(dn)[None, :]
                valid = np.arange(dn)[None, :] < degs[:, None]
                blk = np.zeros((rows, dn), np.int64)
                blk[valid] = srcs_by_dst[np.minimum(pos, rowptr[-1] - 1)[valid]]
                srcs[:rows, co + 1:co + d] = blk
                m = np.full((rows, dn), MASKV, np.float32)
                m[valid] = 0.0
                mask[:rows, co + 1:co + d] = m
        srcs_l.append(srcs)
        idx_l.append(table_row[srcs].astype(np.int32))
        mask_l.append(mask)
    return Prep(perms=perms, groups=groups, C=C, srcs=srcs_l, idx=idx_l,
                mask=mask_l, table_row=table_row)


# --------------------------------------------------------------------------
# Device program
# --------------------------------------------------------------------------

NQ = 4                       # SWDGE queues for the indirect gathers


def indirect_gather_q(gp, out, in_, offset_ap, queue_name):
    """indirect_dma_start (src-indirect gather) pinned to a SWDGE queue."""
    import concourse.mybir as mybir
    from concourse.bass import BassSymbolicTensorAccessPattern

    src_ap = in_
    assert isinstance(src_ap.offset, int) and src_ap.offset == 0
    out_l = gp.lower_ap_dma(out, for_indirect_dma=True)
    in_l = gp.lower_ap_dma(in_, for_indirect_dma=True)
    assert len(in_l) == 1 and len(out_l) == 1
    off_l = gp.lower_ap_dma(offset_ap)
    assert len(off_l) == 1
    off_l = off_l[0]
    in_l.append(off_l)
    ap_shape = src_ap.shape
    coef = 1
    for i in range(1, len(ap_shape)):
        coef *= ap_shape[i]
    in_l[0].dynamic_ap_info = mybir.DynamicAccessPatternInfo(
        c=0,
        actual_ap=out.ap,
        indirect_dim_max_index=ap_shape[0],
        offset_expr=[
            mybir.DynamicAccessPatternOffsetExpr(
                coef=coef,
                aff_expr=mybir.DynamicAccessPatternOffsetExprAffExpr(
                    kind="IndirectArgId", arg_id=1),
            )
        ],
    )
    return gp.add_instruction(
        mybir.InstDMACopy(
            name=gp.bass.get_next_instruction_name(),
            queue=queue_name,
            mode="Copy",
            ins=in_l,
            outs=out_l,
            oob_is_err=True,
            cce_op=mybir.AluOpType.bypass,
        )
    )


def build_program(pp: Prep):
    import concourse.bass as bass
    import concourse.mybir as mybir
    import concourse.tile as tile
    from concourse import bacc

    f32 = mybir.dt.float32
    bf16 = mybir.dt.bfloat16
    i32 = mybir.dt.int32
    A = mybir.AluOpType
    AF = mybir.ActivationFunctionType
    C = pp.C
    NTLMAX = max(g[1] for g in pp.groups)

    nc = bacc.Bacc("TRN2", target_bir_lowering=False, debug=False,
                   num_devices=NC, num_swdge_queues=NQ)

    xgT_t = nc.dram_tensor("xgT", [P, C * P], bf16, kind="ExternalInput")
    xTo_t = nc.dram_tensor("xTo", [P, NTP], bf16, kind="ExternalInput")
    idx_t = nc.dram_tensor("idx", [P, C], i32, kind="ExternalInput")
    mask_t = nc.dram_tensor("mask", [P, C], bf16, kind="ExternalInput")
    rhs0_t = nc.dram_tensor("rhs0", [IN_DIM, HS], bf16, kind="ExternalInput")
    rhs1_t = nc.dram_tensor("rhs1", [HID, HS], bf16, kind="ExternalInput")
    rhsl_t = nc.dram_tensor("rhsl", [HID, CLS], bf16, kind="ExternalInput")
    b0r_t = nc.dram_tensor("b0r", [P, NTLMAX * HID], bf16, kind="ExternalInput")
    b1r_t = nc.dram_tensor("b1r", [P, NTLMAX * HID], bf16, kind="ExternalInput")
    blr_t = nc.dram_tensor("blr", [P, CLS], f32, kind="ExternalInput")
    id_t = nc.dram_tensor("ident", [P, P], bf16, kind="ExternalInput")
    y_t = nc.dram_tensor("y_out", [NTP, CLS], f32, kind="ExternalOutput")

    ha1_loc = nc.dram_tensor("ha1_loc", [NTP, HS], bf16, kind="Internal")
    ha1_full = nc.dram_tensor("ha1_full", [N, HS], bf16, kind="Internal",
                              addr_space="Shared")

    with tile.TileContext(nc) as tc:
        with tc.tile_pool(name="const", bufs=1) as cp, \
             tc.tile_pool(name="xgp", bufs=2) as xgp, \
             tc.tile_pool(name="gp", bufs=3) as gp, \
             tc.tile_pool(name="wp", bufs=3) as wp, \
             tc.tile_pool(name="pmm", bufs=3, space="PSUM") as pmm, \
             tc.tile_pool(name="ptr", bufs=2, space="PSUM") as ptr:

            def load_const(t, shape, dt):
                s = cp.tile(shape, dt, tag=f"c_{t.name}")
                nc.sync.dma_start(s[:], t.ap())
                return s

            rhs0_s = load_const(rhs0_t, [IN_DIM, HS], bf16)
            rhs1_s = load_const(rhs1_t, [HID, HS], bf16)
            rhsl_s = load_const(rhsl_t, [HID, CLS], bf16)
            b0r_s = load_const(b0r_t, [P, NTLMAX * HID], bf16)
            b1r_s = load_const(b1r_t, [P, NTLMAX * HID], bf16)
            blr_s = load_const(blr_t, [P, CLS], f32)
            ident_s = load_const(id_t, [P, P], bf16)
            idx_all = load_const(idx_t, [P, C], i32)
            mask_all = load_const(mask_t, [P, C], bf16)
            xTo_s = load_const(xTo_t, [P, NTP], bf16)

            ha1_sb = cp.tile([P, NT, HS], bf16)
            y_sb = cp.tile([P, NT, CLS], f32)
            d0_all = cp.tile([P, NT], bf16)
            d1_all = cp.tile([P, NT], bf16)
            dmask0 = cp.tile([P, C], bf16)
            dmask1 = cp.tile([P, C], bf16)

            # ---------------- phase A: d0 for own dst nodes ----------------
            for tb in range(0, NT, 7):
                tn = min(7, NT - tb)
                mm = pmm.tile([P, 7, HS], f32, space="PSUM", tag="mmG")
                for t2 in range(tn):
                    t = tb + t2
                    nc.tensor.matmul(out=mm[:, t2, :],
                                     lhsT=xTo_s[:, t * P:(t + 1) * P],
                                     rhs=rhs0_s[:], start=True, stop=True)
                nc.vector.tensor_copy(out=d0_all[:, tb:tb + tn],
                                      in_=mm[:, 0:tn, 65])

            # ---------------- edge phase (shared for both layers) ----------
            def build_dmask(dmask, d_all):
                for (t0, ntl, D, goff) in pp.groups:
                    S = ntl * D
                    nc.vector.tensor_tensor(
                        out=dmask[:, goff:goff + S].rearrange(
                            "p (t d) -> p t d", t=ntl),
                        in0=mask_all[:, goff:goff + S].rearrange(
                            "p (t d) -> p t d", t=ntl),
                        in1=d_all[:, t0:t0 + ntl].to_broadcast([P, ntl, D]),
                        op=A.add)

            def edge_phase(layer, dmask, post):
                for (t0, ntl, D, goff) in pp.groups:
                    S = ntl * D
                    G = gp.tile([P, S, HS], bf16, tag="G")
                    if layer == 0:
                        xg = xgp.tile([P, S * P], bf16, tag="xg")
                        nc.sync.dma_start(
                            xg[:], xgT_t.ap()[:, goff * P:(goff + S) * P])
                        for jb in range(0, S, 7):
                            jn = min(7, S - jb)
                            mm = pmm.tile([P, 7, HS], f32, space="PSUM",
                                          tag="mmG")
                            for j2 in range(jn):
                                j = jb + j2
                                nc.tensor.matmul(
                                    out=mm[:, j2, :],
                                    lhsT=xg[:, j * P:(j + 1) * P],
                                    rhs=rhs0_s[:], start=True, stop=True)
                            nc.scalar.copy(G[:, jb:jb + jn, :],
                                           mm[:, 0:jn, :])
                    else:
                        for j in range(S):
                            if j % D == 0:       # self-loop column: local copy
                                nc.scalar.copy(G[:, j, :],
                                               ha1_sb[:, t0 + j // D, :])
                                continue
                            q = (goff + j) % NQ
                            indirect_gather_q(
                                nc.gpsimd, G[:, j, :], ha1_full.ap(),
                                idx_all[:, goff + j:goff + j + 1],
                                f"qPoolDynamic{q or ''}")

                    # ---- segment softmax over each tile's D slots ----
                    z = wp.tile([P, S], bf16, tag="z")
                    nc.vector.tensor_tensor(out=z[:], in0=G[:, :, 64],
                                            in1=dmask[:, goff:goff + S],
                                            op=A.add)
                    zs = wp.tile([P, S], bf16, tag="zs")
                    nc.vector.tensor_scalar(zs[:], z[:], SLOPE, None,
                                            op0=A.mult)
                    nc.vector.tensor_tensor(out=z[:], in0=z[:], in1=zs[:],
                                            op=A.max)
                    zv = z[:].rearrange("p (t d) -> p t d", t=ntl)
                    nm = wp.tile([P, ntl], bf16, tag="nm")
                    nc.vector.tensor_reduce(out=nm[:], in_=zv,
                                            axis=mybir.AxisListType.X,
                                            op=A.max, negate=True)
                    den = wp.tile([P, ntl], f32, tag="den")
                    for i in range(ntl):
                        nc.scalar.activation(
                            z[:, i * D:(i + 1) * D], z[:, i * D:(i + 1) * D],
                            AF.Exp, bias=nm[:, i:i + 1], scale=1.0,
                            accum_out=den[:, i:i + 1])
                    rden = wp.tile([P, ntl], f32, tag="rden")
                    nc.vector.reciprocal(rden[:], den[:])
                    rb = wp.tile([P, ntl], bf16, tag="rb")
                    nc.vector.tensor_copy(out=rb[:], in_=rden[:])
                    nc.vector.tensor_tensor(
                        out=zv, in0=zv,
                        in1=rb[:].to_broadcast([P, ntl, D]), op=A.mult)
                    # ---- weighted aggregation: G *= alpha, tree-reduce ----
                    nc.vector.tensor_tensor(
                        out=G[:, :, 0:HID], in0=G[:, :, 0:HID],
                        in1=z[:].to_broadcast([P, S, HID]), op=A.mult)
                    G4 = G[:, :, :].rearrange("p (t d) e -> p t d e", t=ntl)
                    h = D
                    while h > 1:
                        a = (h + 1) // 2
                        nc.vector.tensor_tensor(
                            out=G4[:, :, 0:h - a, 0:HID],
                            in0=G4[:, :, 0:h - a, 0:HID],
                            in1=G4[:, :, a:h, 0:HID], op=A.add)
                        h = a
                    agg = G4[:, :, 0, 0:HID]          # [P, ntl, HID]
                    br = (b0r_s if layer == 0 else b1r_s)
                    hb = wp.tile([P, ntl * HID], bf16, tag="hb")
                    hbv = hb[:].rearrange("p (t e) -> p t e", t=ntl)
                    nc.vector.tensor_tensor(
                        out=hbv, in0=agg,
                        in1=br[:, 0:ntl * HID].rearrange(
                            "p (t e) -> p t e", t=ntl), op=A.add)
                    ex = wp.tile([P, ntl * HID], bf16, tag="ex")
                    nc.scalar.activation(ex[:], hb[:], AF.Exp)
                    nc.vector.tensor_scalar(ex[:], ex[:], -1.0, 0.0,
                                            op0=A.add, op1=A.min)
                    rl = wp.tile([P, ntl * HID], bf16, tag="rl")
                    nc.vector.tensor_scalar(rl[:], hb[:], 0.0, None,
                                            op0=A.max)
                    ht = wp.tile([P, ntl * HID], bf16, tag="ht")
                    nc.vector.tensor_tensor(out=ht[:], in0=rl[:], in1=ex[:],
                                            op=A.add)
                    post(t0, ntl, ht)

            # ---------------- posts ----------------
            def transpose_pairs(ntl, ht, consume):
                for i in range(ntl):
                    tp = ptr.tile([HID, P], bf16, space="PSUM", tag="tp")
                    nc.tensor.transpose(out=tp[:, :],
                                        in_=ht[:, i * HID:(i + 1) * HID],
                                        identity=ident_s[:])
                    hT2 = wp.tile([HID, P], bf16, tag="hT2")
                    nc.scalar.copy(hT2[:, :], tp[:, :])
                    consume(i, hT2[:, :])

            def post_l0(t0, ntl, ht):
                def consume(i2, lhsT):
                    t = t0 + i2
                    mm = pmm.tile([P, HS], f32, space="PSUM", tag="mmP")
                    nc.tensor.matmul(out=mm[:, :], lhsT=lhsT, rhs=rhs1_s[:],
                                     start=True, stop=True)
                    nc.scalar.copy(ha1_sb[:, t, :], mm[:, :])
                    nc.vector.tensor_copy(out=d1_all[:, t:t + 1],
                                          in_=mm[:, 65:66])
                transpose_pairs(ntl, ht, consume)

            def post_l1(t0, ntl, ht):
                def consume(i2, lhsT):
                    t = t0 + i2
                    mm = pmm.tile([P, HS], f32, space="PSUM", tag="mmP")
                    nc.tensor.matmul(out=mm[:, 0:CLS], lhsT=lhsT, rhs=rhsl_s[:],
                                     start=True, stop=True)
                    nc.vector.tensor_tensor(out=y_sb[:, t, :], in0=mm[:, 0:CLS],
                                            in1=blr_s[:], op=A.add)
                transpose_pairs(ntl, ht, consume)

            # ---------------- run ----------------
            build_dmask(dmask0, d0_all)
            edge_phase(0, dmask0, post_l0)
            nc.sync.dma_start(
                ha1_loc.ap().rearrange("(t p) e -> p t e", p=P),
                ha1_sb[:, :, :])
            nc.gpsimd.collective_compute(
                "AllGather", A.bypass,
                replica_groups=[list(range(NC))],
                ins=[ha1_loc.ap()[0:SHARD, :]],
                outs=[ha1_full.ap()[0:N, :]],
            )
            build_dmask(dmask1, d1_all)
            edge_phase(1, dmask1, post_l1)
            nc.sync.dma_start(
                y_t.ap().rearrange("(t p) e -> p t e", p=P),
                y_sb[:, :, :])

    nc.compile()
    return nc


# --------------------------------------------------------------------------
# Input staging / output assembly
# --------------------------------------------------------------------------

def make_in_maps(pp: Prep, x, W0, a_s0, a_d0, b0, W1, a_s1, a_d1, b1, Wl, bl):
    bf = ml_dtypes.bfloat16
    x = np.asarray(x, np.float32)
    W0 = np.asarray(W0, np.float32)
    W1 = np.asarray(W1, np.float32)
    Wl = np.asarray(Wl, np.float32)
    NTLMAX = max(g[1] for g in pp.groups)

    rhs0 = np.concatenate(
        [W0, (W0 @ np.asarray(a_s0, np.float32))[:, None],
         (W0 @ np.asarray(a_d0, np.float32))[:, None]], axis=1)
    rhs1 = np.concatenate(
        [W1, (W1 @ np.asarray(a_s1, np.float32))[:, None],
         (W1 @ np.asarray(a_d1, np.float32))[:, None]], axis=1)
    consts = dict(
        rhs0=np.ascontiguousarray(rhs0).astype(bf),
        rhs1=np.ascontiguousarray(rhs1).astype(bf),
        rhsl=np.ascontiguousarray(Wl).astype(bf),
        b0r=np.ascontiguousarray(
            np.tile(np.asarray(b0, np.float32)[None, :], (P, NTLMAX))).astype(bf),
        b1r=np.ascontiguousarray(
            np.tile(np.asarray(b1, np.float32)[None, :], (P, NTLMAX))).astype(bf),
        blr=np.ascontiguousarray(
            np.tile(np.asarray(bl, np.float32)[None, :], (P, 1))),
        ident=np.eye(P, dtype=np.float32).astype(bf),
    )
    xb = x.astype(bf)
    in_maps = []
    for c in range(NC):
        m = dict(consts)
        xg = xb[pp.srcs[c]]                       # [P, C, IN_DIM]
        m["xgT"] = np.ascontiguousarray(
            xg.transpose(2, 1, 0).reshape(IN_DIM, pp.C * P))
        xTo = np.zeros((P, NTP), np.float32)
        xTo[:, :SHARD] = xb[pp.perms[c]].T.astype(np.float32)
        # column t*P+p must hold node at local position t*P+p:
        # perms[c] is already local-order, and xTo columns are local order.
        m["xTo"] = np.ascontiguousarray(xTo).astype(bf)
        m["idx"] = np.ascontiguousarray(pp.idx[c])
        m["mask"] = np.ascontiguousarray(pp.mask[c]).astype(bf)
        in_maps.append(m)
    return in_maps


def assemble_output(pp: Prep, results):
    out = np.zeros((N, CLS), np.float32)
    for c in range(NC):
        out[pp.perms[c]] = results[c]["y_out"][:SHARD]
    return out


_cache = {}
last_result = None


def kernel(**inputs) -> np.ndarray:
    global last_result
    trace = bool(int(os.environ.get("GAT_TRACE", "0")))
    if trace:
        _ensure_profile_hook()
    from concourse.bass_utils import run_bass_kernel_spmd

    ei = np.asarray(inputs["edge_index"])
    key = hash(ei.tobytes())
    if key not in _cache:
        pp = preprocess(ei)
        nc = build_program(pp)
        _cache[key] = (pp, nc)
    pp, nc = _cache[key]

    in_maps = make_in_maps(
        pp, inputs["x"], inputs["W0"], inputs["a_s0"], inputs["a_d0"],
        inputs["b0"], inputs["W1"], inputs["a_s1"], inputs["a_d1"],
        inputs["b1"], inputs["Wl"], inputs["bl"])
    res = run_bass_kernel_spmd(nc, in_maps, core_ids=list(range(NC)),
                               trace=trace)
    last_result = res
    return assemble_output(pp, res.results)


# revision 16
# speedup vs baseline: 1.9250x; 1.0346x over previous
"""Self-contained 8-core Trainium2 Bass kernel for a 2-layer GAT + linear classifier.

v2 design (dst-sharded 1D graph parallelism):
  - Host: add self-loops, degree-sort nodes, deal round-robin to 8 cores.
    Tiles of 128 dst nodes; uniform-degree groups of tiles (all tiles in a
    group share slot width D) so softmax reductions batch into single
    strided DVE instructions.  Pad slots handled by a -3e4 additive mask.
  - Layer 0 needs NO device gather and NO AllGather: the host pre-gathers
    x[src] per edge slot (bf16, transposed per 128-slot column) and the
    device computes [h|s|d] per slot with one PE matmul per column against
    a fused rhs [W0 | W0@a_s0 | W0@a_d0].
  - Layer 1: per-tile outputs [h1|s1|d1] = elu(agg0)@[W1|W1@a_s1|W1@a_d1]
    are written to a packed bf16 table [100000, 66], AllGathered (13 MB),
    then edge rows fetched with per-column indirect DMAs (the only
    HW-correct indexed-DMA form: one offset per partition per call).
  - Classifier fused per tile; one bulk DMA for the table and the output.
"""

import os
import sys
import types
from dataclasses import dataclass

import numpy as np
import ml_dtypes

P = 128
N = 100000
IN_DIM = 128
HID = 64
CLS = 40
NC = 8
HS = 66                      # table/slot row: [h(64) | s | d-or-junk]
SHARD = N // NC              # 12500
NT = (SHARD + P - 1) // P    # 98
NTP = NT * P                 # 12544
GS = 96                      # max slots per group
SLOPE = 0.2
MASKV = -30000.0


def _ensure_profile_hook():
    if "antenv.axon_hooks" in sys.modules:
        return
    try:
        import antenv
        mod = types.ModuleType("antenv.axon_hooks")
        mod._hook = None
        def _set(h):
            mod._hook = h
        def _get():
            return mod._hook
        mod.set_axon_ntff_profile_hook = _set
        mod.get_axon_ntff_profile_hook = _get
        sys.modules["antenv.axon_hooks"] = mod
        antenv.axon_hooks = mod
        from trn_agent_boot.trn_boot import _ntff_profile_via_ctypes
        _set(_ntff_profile_via_ctypes("/opt/axon/libaxon_pjrt.so"))
    except Exception:
        pass


# --------------------------------------------------------------------------
# Host preprocessing
# --------------------------------------------------------------------------

@dataclass
class Prep:
    perms: list          # per core: global node ids in local (row) order
    groups: list         # (t0, ntl, D, coff)
    C: int               # total slot columns
    srcs: list           # per core: [P, C] int64 src node per slot (pads=0)
    idx: list            # per core: [P, C] int32 table row per slot
    mask: list           # per core: [P, C] float32 0 / MASKV
    table_row: np.ndarray
    split_t: int = 0     # tile boundary of AllGather chunk 1


def preprocess(edge_index) -> Prep:
    # Explicit self-loops occupy slot column 0 of every tile (served from
    # local SBUF on device, no gather); only the raw edges get slots 1..deg.
    src_all = np.asarray(edge_index[0]).astype(np.int64)
    dst_all = np.asarray(edge_index[1]).astype(np.int64)
    deg_ns = np.bincount(dst_all, minlength=N).astype(np.int64)
    deg = deg_ns + 1
    order = np.argsort(dst_all, kind="stable")
    srcs_by_dst = src_all[order]
    rowptr = np.zeros(N + 1, np.int64)
    np.cumsum(deg_ns, out=rowptr[1:])

    rank_order = np.argsort(-deg, kind="stable")
    perms = [rank_order[c::NC] for c in range(NC)]

    rows_t = [min(P, SHARD - t * P) for t in range(NT)]
    D = np.zeros(NT, np.int64)
    for c in range(NC):
        dc = deg[perms[c]]
        for t in range(NT):
            D[t] = max(D[t], dc[t * P:t * P + rows_t[t]].max())

    groups = []
    t = 0
    coff = 0
    while t < NT:
        d = int(D[t])
        t0 = t
        t += 1
        while t < NT and int(D[t]) == d and (t - t0 + 1) * d <= GS:
            t += 1
        groups.append((t0, t - t0, d, coff))
        coff += (t - t0) * d
    C = coff

    # AllGather chunk boundary at the first group end covering tile >= 49;
    # table rows are chunk-major so both collective outputs stay contiguous.
    split_t = next(t0 + ntl for (t0, ntl, d, g) in groups if t0 + ntl >= 49)
    SP1 = split_t * P
    SP2 = SHARD - SP1
    rr = np.arange(N)
    cc, pos = rr % NC, rr // NC
    rows = np.where(pos < SP1, cc * SP1 + pos,
                    NC * SP1 + cc * SP2 + (pos - SP1))
    table_row = np.empty(N, np.int64)
    table_row[rank_order] = rows

    srcs_l, idx_l, mask_l = [], [], []
    for c in range(NC):
        srcs = np.zeros((P, C), np.int64)
        mask = np.full((P, C), MASKV, np.float32)
        for (t0, ntl, d, goff) in groups:
            for ti in range(ntl):
                t = t0 + ti
                co = goff + ti * d
                rows = rows_t[t]
                nodes = perms[c][t * P:t * P + rows]
                # column 0: self-loop
                srcs[:rows, co] = nodes
                mask[:rows, co] = 0.0
                # columns 1..d-1: raw edges
                dn = d - 1
                degs = deg_ns[nodes]
                starts = rowptr[nodes]
                pos = starts[:, None] + np.arange(dn)[None, :]
                valid = np.arange(dn)[None, :] < degs[:, None]
                blk = np.zeros((rows, dn), np.int64)
                blk[valid] = srcs_by_dst[np.minimum(pos, rowptr[-1] - 1)[valid]]
                srcs[:rows, co + 1:co + d] = blk
                m = np.full((rows, dn), MASKV, np.float32)
                m[valid] = 0.0
                mask[:rows, co + 1:co + d] = m
        srcs_l.append(srcs)
        idx_l.append(table_row[srcs].astype(np.int32))
        mask_l.append(mask)
    return Prep(perms=perms, groups=groups, C=C, srcs=srcs_l, idx=idx_l,
                mask=mask_l, table_row=table_row, split_t=split_t)


# --------------------------------------------------------------------------
# Device program
# --------------------------------------------------------------------------

NQ = 4                       # SWDGE queues for the indirect gathers


def indirect_gather_q(gp, out, in_, offset_ap, queue_name):
    """indirect_dma_start (src-indirect gather) pinned to a SWDGE queue."""
    import concourse.mybir as mybir
    from concourse.bass import BassSymbolicTensorAccessPattern

    src_ap = in_
    assert isinstance(src_ap.offset, int) and src_ap.offset == 0
    out_l = gp.lower_ap_dma(out, for_indirect_dma=True)
    in_l = gp.lower_ap_dma(in_, for_indirect_dma=True)
    assert len(in_l) == 1 and len(out_l) == 1
    off_l = gp.lower_ap_dma(offset_ap)
    assert len(off_l) == 1
    off_l = off_l[0]
    in_l.append(off_l)
    ap_shape = src_ap.shape
    coef = 1
    for i in range(1, len(ap_shape)):
        coef *= ap_shape[i]
    in_l[0].dynamic_ap_info = mybir.DynamicAccessPatternInfo(
        c=0,
        actual_ap=out.ap,
        indirect_dim_max_index=ap_shape[0],
        offset_expr=[
            mybir.DynamicAccessPatternOffsetExpr(
                coef=coef,
                aff_expr=mybir.DynamicAccessPatternOffsetExprAffExpr(
                    kind="IndirectArgId", arg_id=1),
            )
        ],
    )
    return gp.add_instruction(
        mybir.InstDMACopy(
            name=gp.bass.get_next_instruction_name(),
            queue=queue_name,
            mode="Copy",
            ins=in_l,
            outs=out_l,
            oob_is_err=True,
            cce_op=mybir.AluOpType.bypass,
        )
    )


def build_program(pp: Prep):
    import concourse.bass as bass
    import concourse.mybir as mybir
    import concourse.tile as tile
    from concourse import bacc

    f32 = mybir.dt.float32
    bf16 = mybir.dt.bfloat16
    i32 = mybir.dt.int32
    A = mybir.AluOpType
    AF = mybir.ActivationFunctionType
    C = pp.C
    NTLMAX = max(g[1] for g in pp.groups)

    nc = bacc.Bacc("TRN2", target_bir_lowering=False, debug=False,
                   num_devices=NC, num_swdge_queues=NQ)

    xgT_t = nc.dram_tensor("xgT", [P, C * P], bf16, kind="ExternalInput")
    xTo_t = nc.dram_tensor("xTo", [P, NTP], bf16, kind="ExternalInput")
    idx_t = nc.dram_tensor("idx", [P, C], i32, kind="ExternalInput")
    mask_t = nc.dram_tensor("mask", [P, C], bf16, kind="ExternalInput")
    rhs0_t = nc.dram_tensor("rhs0", [IN_DIM, HS], bf16, kind="ExternalInput")
    rhs1_t = nc.dram_tensor("rhs1", [HID, HS], bf16, kind="ExternalInput")
    rhsl_t = nc.dram_tensor("rhsl", [HID, CLS], bf16, kind="ExternalInput")
    b0r_t = nc.dram_tensor("b0r", [P, NTLMAX * HID], bf16, kind="ExternalInput")
    b1r_t = nc.dram_tensor("b1r", [P, NTLMAX * HID], bf16, kind="ExternalInput")
    blr_t = nc.dram_tensor("blr", [P, CLS], f32, kind="ExternalInput")
    id_t = nc.dram_tensor("ident", [P, P], bf16, kind="ExternalInput")
    y_t = nc.dram_tensor("y_out", [NTP, CLS], f32, kind="ExternalOutput")

    ha1_loc = nc.dram_tensor("ha1_loc", [NTP, HS], bf16, kind="Internal")
    ha1_full = nc.dram_tensor("ha1_full", [N, HS], bf16, kind="Internal",
                              addr_space="Shared")

    with tile.TileContext(nc) as tc:
        with tc.tile_pool(name="const", bufs=1) as cp, \
             tc.tile_pool(name="xgp", bufs=2) as xgp, \
             tc.tile_pool(name="gp", bufs=3) as gp, \
             tc.tile_pool(name="wp", bufs=3) as wp, \
             tc.tile_pool(name="pmm", bufs=3, space="PSUM") as pmm, \
             tc.tile_pool(name="ptr", bufs=2, space="PSUM") as ptr:

            def load_const(t, shape, dt):
                s = cp.tile(shape, dt, tag=f"c_{t.name}")
                nc.sync.dma_start(s[:], t.ap())
                return s

            rhs0_s = load_const(rhs0_t, [IN_DIM, HS], bf16)
            rhs1_s = load_const(rhs1_t, [HID, HS], bf16)
            rhsl_s = load_const(rhsl_t, [HID, CLS], bf16)
            b0r_s = load_const(b0r_t, [P, NTLMAX * HID], bf16)
            b1r_s = load_const(b1r_t, [P, NTLMAX * HID], bf16)
            blr_s = load_const(blr_t, [P, CLS], f32)
            ident_s = load_const(id_t, [P, P], bf16)
            idx_all = load_const(idx_t, [P, C], i32)
            mask_all = load_const(mask_t, [P, C], bf16)
            xTo_s = load_const(xTo_t, [P, NTP], bf16)

            ha1_sb = cp.tile([P, NT, HS], bf16)
            y_sb = cp.tile([P, NT, CLS], f32)
            d0_all = cp.tile([P, NT], bf16)
            d1_all = cp.tile([P, NT], bf16)
            dmask0 = cp.tile([P, C], bf16)
            dmask1 = cp.tile([P, C], bf16)

            # ---------------- phase A: d0 for own dst nodes ----------------
            for tb in range(0, NT, 7):
                tn = min(7, NT - tb)
                mm = pmm.tile([P, 7, HS], f32, space="PSUM", tag="mmG")
                for t2 in range(tn):
                    t = tb + t2
                    nc.tensor.matmul(out=mm[:, t2, :],
                                     lhsT=xTo_s[:, t * P:(t + 1) * P],
                                     rhs=rhs0_s[:], start=True, stop=True)
                nc.vector.tensor_copy(out=d0_all[:, tb:tb + tn],
                                      in_=mm[:, 0:tn, 65])

            # ---------------- edge phase (shared for both layers) ----------
            def build_dmask(dmask, d_all):
                for (t0, ntl, D, goff) in pp.groups:
                    S = ntl * D
                    nc.vector.tensor_tensor(
                        out=dmask[:, goff:goff + S].rearrange(
                            "p (t d) -> p t d", t=ntl),
                        in0=mask_all[:, goff:goff + S].rearrange(
                            "p (t d) -> p t d", t=ntl),
                        in1=d_all[:, t0:t0 + ntl].to_broadcast([P, ntl, D]),
                        op=A.add)

            def edge_phase(layer, dmask, post, after_group=None):
                for (t0, ntl, D, goff) in pp.groups:
                    S = ntl * D
                    G = gp.tile([P, S, HS], bf16, tag="G")
                    if layer == 0:
                        xg = xgp.tile([P, S * P], bf16, tag="xg")
                        nc.sync.dma_start(
                            xg[:], xgT_t.ap()[:, goff * P:(goff + S) * P])
                        for jb in range(0, S, 7):
                            jn = min(7, S - jb)
                            mm = pmm.tile([P, 7, HS], f32, space="PSUM",
                                          tag="mmG")
                            for j2 in range(jn):
                                j = jb + j2
                                nc.tensor.matmul(
                                    out=mm[:, j2, :],
                                    lhsT=xg[:, j * P:(j + 1) * P],
                                    rhs=rhs0_s[:], start=True, stop=True)
                            nc.scalar.copy(G[:, jb:jb + jn, :],
                                           mm[:, 0:jn, :])
                    else:
                        for j in range(S):
                            if j % D == 0:       # self-loop column: local copy
                                nc.scalar.copy(G[:, j, :],
                                               ha1_sb[:, t0 + j // D, :])
                                continue
                            q = (goff + j) % NQ
                            indirect_gather_q(
                                nc.gpsimd, G[:, j, :], ha1_full.ap(),
                                idx_all[:, goff + j:goff + j + 1],
                                f"qPoolDynamic{q or ''}")

                    # ---- segment softmax over each tile's D slots ----
                    z = wp.tile([P, S], bf16, tag="z")
                    nc.vector.tensor_tensor(out=z[:], in0=G[:, :, 64],
                                            in1=dmask[:, goff:goff + S],
                                            op=A.add)
                    zs = wp.tile([P, S], bf16, tag="zs")
                    nc.vector.tensor_scalar(zs[:], z[:], SLOPE, None,
                                            op0=A.mult)
                    nc.vector.tensor_tensor(out=z[:], in0=z[:], in1=zs[:],
                                            op=A.max)
                    zv = z[:].rearrange("p (t d) -> p t d", t=ntl)
                    nm = wp.tile([P, ntl], bf16, tag="nm")
                    nc.vector.tensor_reduce(out=nm[:], in_=zv,
                                            axis=mybir.AxisListType.X,
                                            op=A.max, negate=True)
                    den = wp.tile([P, ntl], f32, tag="den")
                    for i in range(ntl):
                        nc.scalar.activation(
                            z[:, i * D:(i + 1) * D], z[:, i * D:(i + 1) * D],
                            AF.Exp, bias=nm[:, i:i + 1], scale=1.0,
                            accum_out=den[:, i:i + 1])
                    rden = wp.tile([P, ntl], f32, tag="rden")
                    nc.vector.reciprocal(rden[:], den[:])
                    rb = wp.tile([P, ntl], bf16, tag="rb")
                    nc.vector.tensor_copy(out=rb[:], in_=rden[:])
                    nc.vector.tensor_tensor(
                        out=zv, in0=zv,
                        in1=rb[:].to_broadcast([P, ntl, D]), op=A.mult)
                    # ---- weighted aggregation: G *= alpha, tree-reduce ----
                    nc.vector.tensor_tensor(
                        out=G[:, :, 0:HID], in0=G[:, :, 0:HID],
                        in1=z[:].to_broadcast([P, S, HID]), op=A.mult)
                    G4 = G[:, :, :].rearrange("p (t d) e -> p t d e", t=ntl)
                    h = D
                    while h > 1:
                        a = (h + 1) // 2
                        nc.vector.tensor_tensor(
                            out=G4[:, :, 0:h - a, 0:HID],
                            in0=G4[:, :, 0:h - a, 0:HID],
                            in1=G4[:, :, a:h, 0:HID], op=A.add)
                        h = a
                    agg = G4[:, :, 0, 0:HID]          # [P, ntl, HID]
                    br = (b0r_s if layer == 0 else b1r_s)
                    hb = wp.tile([P, ntl * HID], bf16, tag="hb")
                    hbv = hb[:].rearrange("p (t e) -> p t e", t=ntl)
                    nc.vector.tensor_tensor(
                        out=hbv, in0=agg,
                        in1=br[:, 0:ntl * HID].rearrange(
                            "p (t e) -> p t e", t=ntl), op=A.add)
                    ex = wp.tile([P, ntl * HID], bf16, tag="ex")
                    nc.scalar.activation(ex[:], hb[:], AF.Exp)
                    nc.vector.tensor_scalar(ex[:], ex[:], -1.0, 0.0,
                                            op0=A.add, op1=A.min)
                    rl = wp.tile([P, ntl * HID], bf16, tag="rl")
                    nc.vector.tensor_scalar(rl[:], hb[:], 0.0, None,
                                            op0=A.max)
                    ht = wp.tile([P, ntl * HID], bf16, tag="ht")
                    nc.vector.tensor_tensor(out=ht[:], in0=rl[:], in1=ex[:],
                                            op=A.add)
                    post(t0, ntl, ht)
                    if after_group is not None:
                        after_group(t0 + ntl)

            # ---------------- posts ----------------
            def transpose_pairs(ntl, ht, consume):
                for i in range(ntl):
                    tp = ptr.tile([HID, P], bf16, space="PSUM", tag="tp")
                    nc.tensor.transpose(out=tp[:, :],
                                        in_=ht[:, i * HID:(i + 1) * HID],
                                        identity=ident_s[:])
                    hT2 = wp.tile([HID, P], bf16, tag="hT2")
                    nc.scalar.copy(hT2[:, :], tp[:, :])
                    consume(i, hT2[:, :])

            def post_l0(t0, ntl, ht):
                def consume(i2, lhsT):
                    t = t0 + i2
                    mm = pmm.tile([P, HS], f32, space="PSUM", tag="mmP")
                    nc.tensor.matmul(out=mm[:, :], lhsT=lhsT, rhs=rhs1_s[:],
                                     start=True, stop=True)
                    nc.scalar.copy(ha1_sb[:, t, :], mm[:, :])
                    nc.vector.tensor_copy(out=d1_all[:, t:t + 1],
                                          in_=mm[:, 65:66])
                transpose_pairs(ntl, ht, consume)

            def post_l1(t0, ntl, ht):
                def consume(i2, lhsT):
                    t = t0 + i2
                    mm = pmm.tile([P, HS], f32, space="PSUM", tag="mmP")
                    nc.tensor.matmul(out=mm[:, 0:CLS], lhsT=lhsT, rhs=rhsl_s[:],
                                     start=True, stop=True)
                    nc.vector.tensor_tensor(out=y_sb[:, t, :], in0=mm[:, 0:CLS],
                                            in1=blr_s[:], op=A.add)
                transpose_pairs(ntl, ht, consume)

            # ---------------- run ----------------
            SPT = pp.split_t
            SP1 = SPT * P
            SP2 = SHARD - SP1

            def after_group_l0(tend):
                if tend != SPT:
                    return
                nc.sync.dma_start(
                    ha1_loc.ap()[0:SP1, :].rearrange("(t p) e -> p t e", p=P),
                    ha1_sb[:, 0:SPT, :])
                nc.gpsimd.collective_compute(
                    "AllGather", A.bypass,
                    replica_groups=[list(range(NC))],
                    ins=[ha1_loc.ap()[0:SP1, :]],
                    outs=[ha1_full.ap()[0:NC * SP1, :]],
                )

            build_dmask(dmask0, d0_all)
            edge_phase(0, dmask0, post_l0, after_group=after_group_l0)
            nc.sync.dma_start(
                ha1_loc.ap()[SP1:NTP, :].rearrange("(t p) e -> p t e", p=P),
                ha1_sb[:, SPT:NT, :])
            nc.gpsimd.collective_compute(
                "AllGather", A.bypass,
                replica_groups=[list(range(NC))],
                ins=[ha1_loc.ap()[SP1:SHARD, :]],
                outs=[ha1_full.ap()[NC * SP1:N, :]],
            )
            build_dmask(dmask1, d1_all)
            edge_phase(1, dmask1, post_l1)
            nc.sync.dma_start(
                y_t.ap().rearrange("(t p) e -> p t e", p=P),
                y_sb[:, :, :])

    nc.compile()
    return nc


# --------------------------------------------------------------------------
# Input staging / output assembly
# --------------------------------------------------------------------------

def make_in_maps(pp: Prep, x, W0, a_s0, a_d0, b0, W1, a_s1, a_d1, b1, Wl, bl):
    bf = ml_dtypes.bfloat16
    x = np.asarray(x, np.float32)
    W0 = np.asarray(W0, np.float32)
    W1 = np.asarray(W1, np.float32)
    Wl = np.asarray(Wl, np.float32)
    NTLMAX = max(g[1] for g in pp.groups)

    rhs0 = np.concatenate(
        [W0, (W0 @ np.asarray(a_s0, np.float32))[:, None],
         (W0 @ np.asarray(a_d0, np.float32))[:, None]], axis=1)
    rhs1 = np.concatenate(
        [W1, (W1 @ np.asarray(a_s1, np.float32))[:, None],
         (W1 @ np.asarray(a_d1, np.float32))[:, None]], axis=1)
    consts = dict(
        rhs0=np.ascontiguousarray(rhs0).astype(bf),
        rhs1=np.ascontiguousarray(rhs1).astype(bf),
        rhsl=np.ascontiguousarray(Wl).astype(bf),
        b0r=np.ascontiguousarray(
            np.tile(np.asarray(b0, np.float32)[None, :], (P, NTLMAX))).astype(bf),
        b1r=np.ascontiguousarray(
            np.tile(np.asarray(b1, np.float32)[None, :], (P, NTLMAX))).astype(bf),
        blr=np.ascontiguousarray(
            np.tile(np.asarray(bl, np.float32)[None, :], (P, 1))),
        ident=np.eye(P, dtype=np.float32).astype(bf),
    )
    xb = x.astype(bf)
    in_maps = []
    for c in range(NC):
        m = dict(consts)
        xg = xb[pp.srcs[c]]                       # [P, C, IN_DIM]
        m["xgT"] = np.ascontiguousarray(
            xg.transpose(2, 1, 0).reshape(IN_DIM, pp.C * P))
        xTo = np.zeros((P, NTP), np.float32)
        xTo[:, :SHARD] = xb[pp.perms[c]].T.astype(np.float32)
        # column t*P+p must hold node at local position t*P+p:
        # perms[c] is already local-order, and xTo columns are local order.
        m["xTo"] = np.ascontiguousarray(xTo).astype(bf)
        m["idx"] = np.ascontiguousarray(pp.idx[c])
        m["mask"] = np.ascontiguousarray(pp.mask[c]).astype(bf)
        in_maps.append(m)
    return in_maps


def assemble_output(pp: Prep, results):
    out = np.zeros((N, CLS), np.float32)
    for c in range(NC):
        out[pp.perms[c]] = results[c]["y_out"][:SHARD]
    return out


_cache = {}
last_result = None


def kernel(**inputs) -> np.ndarray:
    global last_result
    trace = bool(int(os.environ.get("GAT_TRACE", "0")))
    if trace:
        _ensure_profile_hook()
    from concourse.bass_utils import run_bass_kernel_spmd

    ei = np.asarray(inputs["edge_index"])
    key = hash(ei.tobytes())
    if key not in _cache:
        pp = preprocess(ei)
        nc = build_program(pp)
        _cache[key] = (pp, nc)
    pp, nc = _cache[key]

    in_maps = make_in_maps(
        pp, inputs["x"], inputs["W0"], inputs["a_s0"], inputs["a_d0"],
        inputs["b0"], inputs["W1"], inputs["a_s1"], inputs["a_d1"],
        inputs["b1"], inputs["Wl"], inputs["bl"])
    res = run_bass_kernel_spmd(nc, in_maps, core_ids=list(range(NC)),
                               trace=trace)
    last_result = res
    return assemble_output(pp, res.results)


# revision 18
# speedup vs baseline: 1.9374x; 1.0065x over previous
"""Self-contained 8-core Trainium2 Bass kernel for a 2-layer GAT + linear classifier.

v2 design (dst-sharded 1D graph parallelism):
  - Host: add self-loops, degree-sort nodes, deal round-robin to 8 cores.
    Tiles of 128 dst nodes; uniform-degree groups of tiles (all tiles in a
    group share slot width D) so softmax reductions batch into single
    strided DVE instructions.  Pad slots handled by a -3e4 additive mask.
  - Layer 0 needs NO device gather and NO AllGather: the host pre-gathers
    x[src] per edge slot (bf16, transposed per 128-slot column) and the
    device computes [h|s|d] per slot with one PE matmul per column against
    a fused rhs [W0 | W0@a_s0 | W0@a_d0].
  - Layer 1: per-tile outputs [h1|s1|d1] = elu(agg0)@[W1|W1@a_s1|W1@a_d1]
    are written to a packed bf16 table [100000, 66], AllGathered (13 MB),
    then edge rows fetched with per-column indirect DMAs (the only
    HW-correct indexed-DMA form: one offset per partition per call).
  - Classifier fused per tile; one bulk DMA for the table and the output.
"""

import os
import sys
import types
from dataclasses import dataclass

import numpy as np
import ml_dtypes

P = 128
N = 100000
IN_DIM = 128
HID = 64
CLS = 40
NC = 8
HS = 66                      # table/slot row: [h(64) | s | d-or-junk]
SHARD = N // NC              # 12500
NT = (SHARD + P - 1) // P    # 98
NTP = NT * P                 # 12544
GS = 96                      # max slots per group
SLOPE = 0.2
MASKV = -30000.0


def _ensure_profile_hook():
    if "antenv.axon_hooks" in sys.modules:
        return
    try:
        import antenv
        mod = types.ModuleType("antenv.axon_hooks")
        mod._hook = None
        def _set(h):
            mod._hook = h
        def _get():
            return mod._hook
        mod.set_axon_ntff_profile_hook = _set
        mod.get_axon_ntff_profile_hook = _get
        sys.modules["antenv.axon_hooks"] = mod
        antenv.axon_hooks = mod
        from trn_agent_boot.trn_boot import _ntff_profile_via_ctypes
        _set(_ntff_profile_via_ctypes("/opt/axon/libaxon_pjrt.so"))
    except Exception:
        pass


# --------------------------------------------------------------------------
# Host preprocessing
# --------------------------------------------------------------------------

@dataclass
class Prep:
    perms: list          # per core: global node ids in local (row) order
    groups: list         # (t0, ntl, D, coff)
    C: int               # total slot columns
    srcs: list           # per core: [P, C] int64 src node per slot (pads=0)
    idx: list            # per core: [P, C] int32 table row per slot
    mask: list           # per core: [P, C] float32 0 / MASKV
    table_row: np.ndarray
    split_t: int = 0     # tile boundary of AllGather chunk 1


def preprocess(edge_index) -> Prep:
    # Explicit self-loops occupy slot column 0 of every tile (served from
    # local SBUF on device, no gather); only the raw edges get slots 1..deg.
    src_all = np.asarray(edge_index[0]).astype(np.int64)
    dst_all = np.asarray(edge_index[1]).astype(np.int64)
    deg_ns = np.bincount(dst_all, minlength=N).astype(np.int64)
    deg = deg_ns + 1
    order = np.argsort(dst_all, kind="stable")
    srcs_by_dst = src_all[order]
    rowptr = np.zeros(N + 1, np.int64)
    np.cumsum(deg_ns, out=rowptr[1:])

    rank_order = np.argsort(-deg, kind="stable")
    perms = [rank_order[c::NC] for c in range(NC)]

    rows_t = [min(P, SHARD - t * P) for t in range(NT)]
    D = np.zeros(NT, np.int64)
    for c in range(NC):
        dc = deg[perms[c]]
        for t in range(NT):
            D[t] = max(D[t], dc[t * P:t * P + rows_t[t]].max())

    groups = []
    t = 0
    coff = 0
    while t < NT:
        d = int(D[t])
        t0 = t
        t += 1
        while t < NT and int(D[t]) == d and (t - t0 + 1) * d <= GS:
            t += 1
        groups.append((t0, t - t0, d, coff))
        coff += (t - t0) * d
    C = coff

    # AllGather chunk boundary at the first group end covering tile >= 49;
    # table rows are chunk-major so both collective outputs stay contiguous.
    split_t = next(t0 + ntl for (t0, ntl, d, g) in groups if t0 + ntl >= 49)
    SP1 = split_t * P
    SP2 = SHARD - SP1
    rr = np.arange(N)
    cc, pos = rr % NC, rr // NC
    rows = np.where(pos < SP1, cc * SP1 + pos,
                    NC * SP1 + cc * SP2 + (pos - SP1))
    table_row = np.empty(N, np.int64)
    table_row[rank_order] = rows

    srcs_l, idx_l, mask_l = [], [], []
    for c in range(NC):
        srcs = np.zeros((P, C), np.int64)
        mask = np.full((P, C), MASKV, np.float32)
        for (t0, ntl, d, goff) in groups:
            for ti in range(ntl):
                t = t0 + ti
                co = goff + ti * d
                rows = rows_t[t]
                nodes = perms[c][t * P:t * P + rows]
                # column 0: self-loop
                srcs[:rows, co] = nodes
                mask[:rows, co] = 0.0
                # columns 1..d-1: raw edges
                dn = d - 1
                degs = deg_ns[nodes]
                starts = rowptr[nodes]
                pos = starts[:, None] + np.arange(dn)[None, :]
                valid = np.arange(dn)[None, :] < degs[:, None]
                blk = np.zeros((rows, dn), np.int64)
                blk[valid] = srcs_by_dst[np.minimum(pos, rowptr[-1] - 1)[valid]]
                srcs[:rows, co + 1:co + d] = blk
                m = np.full((rows, dn), MASKV, np.float32)
                m[valid] = 0.0
                mask[:rows, co + 1:co + d] = m
        srcs_l.append(srcs)
        idx_l.append(table_row[srcs].astype(np.int32))
        mask_l.append(mask)
    return Prep(perms=perms, groups=groups, C=C, srcs=srcs_l, idx=idx_l,
                mask=mask_l, table_row=table_row, split_t=split_t)


# --------------------------------------------------------------------------
# Device program
# --------------------------------------------------------------------------

NQ = 4                       # SWDGE queues for the indirect gathers


def indirect_gather_q(gp, out, in_, offset_ap, queue_name):
    """indirect_dma_start (src-indirect gather) pinned to a SWDGE queue."""
    import concourse.mybir as mybir
    from concourse.bass import BassSymbolicTensorAccessPattern

    src_ap = in_
    assert isinstance(src_ap.offset, int) and src_ap.offset == 0
    out_l = gp.lower_ap_dma(out, for_indirect_dma=True)
    in_l = gp.lower_ap_dma(in_, for_indirect_dma=True)
    assert len(in_l) == 1 and len(out_l) == 1
    off_l = gp.lower_ap_dma(offset_ap)
    assert len(off_l) == 1
    off_l = off_l[0]
    in_l.append(off_l)
    ap_shape = src_ap.shape
    coef = 1
    for i in range(1, len(ap_shape)):
        coef *= ap_shape[i]
    in_l[0].dynamic_ap_info = mybir.DynamicAccessPatternInfo(
        c=0,
        actual_ap=out.ap,
        indirect_dim_max_index=ap_shape[0],
        offset_expr=[
            mybir.DynamicAccessPatternOffsetExpr(
                coef=coef,
                aff_expr=mybir.DynamicAccessPatternOffsetExprAffExpr(
                    kind="IndirectArgId", arg_id=1),
            )
        ],
    )
    return gp.add_instruction(
        mybir.InstDMACopy(
            name=gp.bass.get_next_instruction_name(),
            queue=queue_name,
            mode="Copy",
            ins=in_l,
            outs=out_l,
            oob_is_err=True,
            cce_op=mybir.AluOpType.bypass,
        )
    )


def build_program(pp: Prep):
    import concourse.bass as bass
    import concourse.mybir as mybir
    import concourse.tile as tile
    from concourse import bacc

    f32 = mybir.dt.float32
    bf16 = mybir.dt.bfloat16
    i32 = mybir.dt.int32
    A = mybir.AluOpType
    AF = mybir.ActivationFunctionType
    C = pp.C
    NTLMAX = max(g[1] for g in pp.groups)

    nc = bacc.Bacc("TRN2", target_bir_lowering=False, debug=False,
                   num_devices=NC, num_swdge_queues=NQ)

    xgT_t = nc.dram_tensor("xgT", [P, C * P], bf16, kind="ExternalInput")
    xTo_t = nc.dram_tensor("xTo", [P, NTP], bf16, kind="ExternalInput")
    idx_t = nc.dram_tensor("idx", [P, C], i32, kind="ExternalInput")
    mask_t = nc.dram_tensor("mask", [P, C], bf16, kind="ExternalInput")
    rhs0_t = nc.dram_tensor("rhs0", [IN_DIM, HS], bf16, kind="ExternalInput")
    rhs1_t = nc.dram_tensor("rhs1", [HID, HS], bf16, kind="ExternalInput")
    rhsl_t = nc.dram_tensor("rhsl", [HID, CLS], bf16, kind="ExternalInput")
    b0r_t = nc.dram_tensor("b0r", [P, NTLMAX * HID], bf16, kind="ExternalInput")
    b1r_t = nc.dram_tensor("b1r", [P, NTLMAX * HID], bf16, kind="ExternalInput")
    blr_t = nc.dram_tensor("blr", [P, CLS], f32, kind="ExternalInput")
    id_t = nc.dram_tensor("ident", [P, P], bf16, kind="ExternalInput")
    y_t = nc.dram_tensor("y_out", [NTP, CLS], f32, kind="ExternalOutput")

    ha1_loc = nc.dram_tensor("ha1_loc", [NTP, HS], bf16, kind="Internal")
    ha1_full = nc.dram_tensor("ha1_full", [N, HS], bf16, kind="Internal",
                              addr_space="Shared")

    with tile.TileContext(nc) as tc:
        with tc.tile_pool(name="const", bufs=1) as cp, \
             tc.tile_pool(name="xgp", bufs=2) as xgp, \
             tc.tile_pool(name="gp", bufs=3) as gp, \
             tc.tile_pool(name="wp", bufs=3) as wp, \
             tc.tile_pool(name="pmm", bufs=3, space="PSUM") as pmm, \
             tc.tile_pool(name="ptr", bufs=2, space="PSUM") as ptr:

            def load_const(t, shape, dt):
                s = cp.tile(shape, dt, tag=f"c_{t.name}")
                nc.sync.dma_start(s[:], t.ap())
                return s

            rhs0_s = load_const(rhs0_t, [IN_DIM, HS], bf16)
            rhs1_s = load_const(rhs1_t, [HID, HS], bf16)
            rhsl_s = load_const(rhsl_t, [HID, CLS], bf16)
            b0r_s = load_const(b0r_t, [P, NTLMAX * HID], bf16)
            b1r_s = load_const(b1r_t, [P, NTLMAX * HID], bf16)
            blr_s = load_const(blr_t, [P, CLS], f32)
            ident_s = load_const(id_t, [P, P], bf16)
            idx_all = load_const(idx_t, [P, C], i32)
            mask_all = load_const(mask_t, [P, C], bf16)
            xTo_s = load_const(xTo_t, [P, NTP], bf16)

            ha1_sb = cp.tile([P, NT, HS], bf16)
            y_sb = cp.tile([P, NT, CLS], f32)
            d0_all = cp.tile([P, NT], bf16)
            d1_all = cp.tile([P, NT], bf16)
            dmask0 = cp.tile([P, C], bf16)
            dmask1 = cp.tile([P, C], bf16)

            # ---------------- phase A: d0 for own dst nodes ----------------
            for tb in range(0, NT, 7):
                tn = min(7, NT - tb)
                mm = pmm.tile([P, 7, HS], f32, space="PSUM", tag="mmG")
                for t2 in range(tn):
                    t = tb + t2
                    nc.tensor.matmul(out=mm[:, t2, :],
                                     lhsT=xTo_s[:, t * P:(t + 1) * P],
                                     rhs=rhs0_s[:], start=True, stop=True)
                nc.vector.tensor_copy(out=d0_all[:, tb:tb + tn],
                                      in_=mm[:, 0:tn, 65])

            # ---------------- edge phase (shared for both layers) ----------
            def build_dmask(dmask, d_all):
                for (t0, ntl, D, goff) in pp.groups:
                    S = ntl * D
                    nc.vector.tensor_tensor(
                        out=dmask[:, goff:goff + S].rearrange(
                            "p (t d) -> p t d", t=ntl),
                        in0=mask_all[:, goff:goff + S].rearrange(
                            "p (t d) -> p t d", t=ntl),
                        in1=d_all[:, t0:t0 + ntl].to_broadcast([P, ntl, D]),
                        op=A.add)

            def edge_phase(layer, dmask, post, after_group=None):
                for (t0, ntl, D, goff) in pp.groups:
                    S = ntl * D
                    G = gp.tile([P, S, HS], bf16, tag="G")
                    if layer == 0:
                        xg = xgp.tile([P, S * P], bf16, tag="xg")
                        nc.sync.dma_start(
                            xg[:], xgT_t.ap()[:, goff * P:(goff + S) * P])
                        for jb in range(0, S, 7):
                            jn = min(7, S - jb)
                            mm = pmm.tile([P, 7, HS], f32, space="PSUM",
                                          tag="mmG")
                            for j2 in range(jn):
                                j = jb + j2
                                nc.tensor.matmul(
                                    out=mm[:, j2, :],
                                    lhsT=xg[:, j * P:(j + 1) * P],
                                    rhs=rhs0_s[:], start=True, stop=True)
                            nc.scalar.copy(G[:, jb:jb + jn, :],
                                           mm[:, 0:jn, :])
                    else:
                        for j in range(S):
                            if j % D == 0:       # self-loop column: local copy
                                nc.scalar.copy(G[:, j, :],
                                               ha1_sb[:, t0 + j // D, :])
                                continue
                            q = (goff + j) % NQ
                            indirect_gather_q(
                                nc.gpsimd, G[:, j, :], ha1_full.ap(),
                                idx_all[:, goff + j:goff + j + 1],
                                f"qPoolDynamic{q or ''}")

                    # ---- segment softmax over each tile's D slots ----
                    z = wp.tile([P, S], bf16, tag="z")
                    nc.vector.tensor_tensor(out=z[:], in0=G[:, :, 64],
                                            in1=dmask[:, goff:goff + S],
                                            op=A.add)
                    zs = wp.tile([P, S], bf16, tag="zs")
                    nc.vector.tensor_scalar(zs[:], z[:], SLOPE, None,
                                            op0=A.mult)
                    nc.vector.tensor_tensor(out=z[:], in0=z[:], in1=zs[:],
                                            op=A.max)
                    zv = z[:].rearrange("p (t d) -> p t d", t=ntl)
                    nm = wp.tile([P, ntl], bf16, tag="nm")
                    nc.vector.tensor_reduce(out=nm[:], in_=zv,
                                            axis=mybir.AxisListType.X,
                                            op=A.max, negate=True)
                    den = wp.tile([P, ntl], f32, tag="den")
                    for i in range(ntl):
                        nc.scalar.activation(
                            z[:, i * D:(i + 1) * D], z[:, i * D:(i + 1) * D],
                            AF.Exp, bias=nm[:, i:i + 1], scale=1.0,
                            accum_out=den[:, i:i + 1])
                    rden = wp.tile([P, ntl], f32, tag="rden")
                    nc.vector.reciprocal(rden[:], den[:])
                    rb = wp.tile([P, ntl], bf16, tag="rb")
                    nc.vector.tensor_copy(out=rb[:], in_=rden[:])
                    nc.vector.tensor_tensor(
                        out=zv, in0=zv,
                        in1=rb[:].to_broadcast([P, ntl, D]), op=A.mult)
                    # ---- weighted aggregation: G *= alpha, tree-reduce ----
                    nc.vector.tensor_tensor(
                        out=G[:, :, 0:HID], in0=G[:, :, 0:HID],
                        in1=z[:].to_broadcast([P, S, HID]), op=A.mult)
                    G4 = G[:, :, :].rearrange("p (t d) e -> p t d e", t=ntl)
                    h = D
                    while h > 1:
                        a = (h + 1) // 2
                        nc.vector.tensor_tensor(
                            out=G4[:, :, 0:h - a, 0:HID],
                            in0=G4[:, :, 0:h - a, 0:HID],
                            in1=G4[:, :, a:h, 0:HID], op=A.add)
                        h = a
                    agg = G4[:, :, 0, 0:HID]          # [P, ntl, HID]
                    br = (b0r_s if layer == 0 else b1r_s)
                    hb = wp.tile([P, ntl * HID], bf16, tag="hb")
                    hbv = hb[:].rearrange("p (t e) -> p t e", t=ntl)
                    nc.vector.tensor_tensor(
                        out=hbv, in0=agg,
                        in1=br[:, 0:ntl * HID].rearrange(
                            "p (t e) -> p t e", t=ntl), op=A.add)
                    ex = wp.tile([P, ntl * HID], bf16, tag="ex")
                    nc.scalar.activation(ex[:], hb[:], AF.Exp)
                    nc.vector.tensor_scalar(ex[:], ex[:], -1.0, 0.0,
                                            op0=A.add, op1=A.min)
                    rl = wp.tile([P, ntl * HID], bf16, tag="rl")
                    nc.vector.tensor_scalar(rl[:], hb[:], 0.0, None,
                                            op0=A.max)
                    ht = wp.tile([P, ntl * HID], bf16, tag="ht")
                    nc.vector.tensor_tensor(out=ht[:], in0=rl[:], in1=ex[:],
                                            op=A.add)
                    post(t0, ntl, ht)
                    if after_group is not None:
                        after_group(t0 + ntl)

            # ---------------- posts ----------------
            def transpose_pairs(ntl, ht, consume):
                for i in range(ntl):
                    tp = ptr.tile([HID, P], bf16, space="PSUM", tag="tp")
                    nc.tensor.transpose(out=tp[:, :],
                                        in_=ht[:, i * HID:(i + 1) * HID],
                                        identity=ident_s[:])
                    hT2 = wp.tile([HID, P], bf16, tag="hT2")
                    nc.scalar.copy(hT2[:, :], tp[:, :])
                    consume(i, hT2[:, :])

            def post_l0(t0, ntl, ht):
                def consume(i2, lhsT):
                    t = t0 + i2
                    mm = pmm.tile([P, HS], f32, space="PSUM", tag="mmP")
                    nc.tensor.matmul(out=mm[:, :], lhsT=lhsT, rhs=rhs1_s[:],
                                     start=True, stop=True)
                    nc.scalar.copy(ha1_sb[:, t, :], mm[:, :])
                    nc.vector.tensor_copy(out=d1_all[:, t:t + 1],
                                          in_=mm[:, 65:66])
                transpose_pairs(ntl, ht, consume)

            def post_l1(t0, ntl, ht):
                def consume(i2, lhsT):
                    t = t0 + i2
                    mm = pmm.tile([P, HS], f32, space="PSUM", tag="mmP")
                    nc.tensor.matmul(out=mm[:, 0:CLS], lhsT=lhsT, rhs=rhsl_s[:],
                                     start=True, stop=True)
                    nc.vector.tensor_tensor(out=y_sb[:, t, :], in0=mm[:, 0:CLS],
                                            in1=blr_s[:], op=A.add)
                transpose_pairs(ntl, ht, consume)

            # ---------------- run ----------------
            SPT = pp.split_t
            SP1 = SPT * P
            SP2 = SHARD - SP1

            def after_group_l0(tend):
                if tend != SPT:
                    return
                nc.sync.dma_start(
                    ha1_loc.ap()[0:SP1, :].rearrange("(t p) e -> p t e", p=P),
                    ha1_sb[:, 0:SPT, :])
                nc.gpsimd.collective_compute(
                    "AllGather", A.bypass,
                    replica_groups=[list(range(NC))],
                    ins=[ha1_loc.ap()[0:SP1, :]],
                    outs=[ha1_full.ap()[0:NC * SP1, :]],
                )

            build_dmask(dmask0, d0_all)
            edge_phase(0, dmask0, post_l0, after_group=after_group_l0)
            nc.sync.dma_start(
                ha1_loc.ap()[SP1:NTP, :].rearrange("(t p) e -> p t e", p=P),
                ha1_sb[:, SPT:NT, :])
            nc.gpsimd.collective_compute(
                "AllGather", A.bypass,
                replica_groups=[list(range(NC))],
                ins=[ha1_loc.ap()[SP1:SHARD, :]],
                outs=[ha1_full.ap()[NC * SP1:N, :]],
            )
            build_dmask(dmask1, d1_all)
            edge_phase(1, dmask1, post_l1)
            nc.sync.dma_start(
                y_t.ap().rearrange("(t p) e -> p t e", p=P),
                y_sb[:, :, :])

    nc.compile()
    return nc


# --------------------------------------------------------------------------
# Input staging / output assembly
# --------------------------------------------------------------------------

def make_in_maps(pp: Prep, x, W0, a_s0, a_d0, b0, W1, a_s1, a_d1, b1, Wl, bl):
    bf = ml_dtypes.bfloat16
    x = np.asarray(x, np.float32)
    W0 = np.asarray(W0, np.float32)
    W1 = np.asarray(W1, np.float32)
    Wl = np.asarray(Wl, np.float32)
    NTLMAX = max(g[1] for g in pp.groups)

    rhs0 = np.concatenate(
        [W0, (W0 @ np.asarray(a_s0, np.float32))[:, None],
         (W0 @ np.asarray(a_d0, np.float32))[:, None]], axis=1)
    rhs1 = np.concatenate(
        [W1, (W1 @ np.asarray(a_s1, np.float32))[:, None],
         (W1 @ np.asarray(a_d1, np.float32))[:, None]], axis=1)
    consts = dict(
        rhs0=np.ascontiguousarray(rhs0).astype(bf),
        rhs1=np.ascontiguousarray(rhs1).astype(bf),
        rhsl=np.ascontiguousarray(Wl).astype(bf),
        b0r=np.ascontiguousarray(
            np.tile(np.asarray(b0, np.float32)[None, :], (P, NTLMAX))).astype(bf),
        b1r=np.ascontiguousarray(
            np.tile(np.asarray(b1, np.float32)[None, :], (P, NTLMAX))).astype(bf),
        blr=np.ascontiguousarray(
            np.tile(np.asarray(bl, np.float32)[None, :], (P, 1))),
        ident=np.eye(P, dtype=np.float32).astype(bf),
    )
    xb = x.astype(bf)
    in_maps = []
    for c in range(NC):
        m = dict(consts)
        xg = xb[pp.srcs[c]]                       # [P, C, IN_DIM]
        m["xgT"] = np.ascontiguousarray(
            xg.transpose(2, 1, 0).reshape(IN_DIM, pp.C * P))
        xTo = np.zeros((P, NTP), np.float32)
        xTo[:, :SHARD] = xb[pp.perms[c]].T.astype(np.float32)
        # column t*P+p must hold node at local position t*P+p:
        # perms[c] is already local-order, and xTo columns are local order.
        m["xTo"] = np.ascontiguousarray(xTo).astype(bf)
        m["idx"] = np.ascontiguousarray(pp.idx[c])
        m["mask"] = np.ascontiguousarray(pp.mask[c]).astype(bf)
        in_maps.append(m)
    return in_maps


def assemble_output(pp: Prep, results):
    out = np.zeros((N, CLS), np.float32)
    for c in range(NC):
        out[pp.perms[c]] = results[c]["y_out"][:SHARD]
    return out


_cache = {}
last_result = None


def kernel(**inputs) -> np.ndarray:
    global last_result
    trace = bool(int(os.environ.get("GAT_TRACE", "0")))
    if trace:
        _ensure_profile_hook()
    from concourse.bass_utils import run_bass_kernel_spmd

    ei = np.asarray(inputs["edge_index"])
    key = hash(ei.tobytes())
    if key not in _cache:
        pp = preprocess(ei)
        nc = build_program(pp)
        _cache[key] = (pp, nc)
    pp, nc = _cache[key]

    in_maps = make_in_maps(
        pp, inputs["x"], inputs["W0"], inputs["a_s0"], inputs["a_d0"],
        inputs["b0"], inputs["W1"], inputs["a_s1"], inputs["a_d1"],
        inputs["b1"], inputs["Wl"], inputs["bl"])
    res = run_bass_kernel_spmd(nc, in_maps, core_ids=list(range(NC)),
                               trace=trace)
    last_result = res
    return assemble_output(pp, res.results)
